# revision 16
# baseline (speedup 1.0000x reference)
"""DGNN message-passing kernel for 8 TRN2 NeuronCores.

Row-shards the n dimension across 8 cores. Algebraic restructuring: the n x n
Gram matrices (temp11 = yw@yw.T - eps*z1w@z1w.T - ...) are never materialized.
Instead, for each sigmoid argument:

    (temp11 @ t)[rows_c, :] = sum_x coef_x * xw_x[rows_c, :] @ S_{x,t}
    with S_{x,t} = xw_x.T @ t = sum_c xw_x[rows_c].T @ t[rows_c]   (f x f)

The S matrices are computed as per-core block partials and AllReduced (f x f
each). The only O(n^2) matmuls left are adj @ z1 / adj1 @ z2, block-rows per
core. Between layers, z1/z2 row-blocks are AllGathered (y is only ever needed
block-wise). All heavy matmuls run in bf16 (fp32 PSUM accumulation); the
sigmoids saturate, final rel err vs fp64 reference ~1e-5.
"""

import numpy as np
import ml_dtypes

import concourse.bass as bass
import concourse.mybir as mybir
import concourse.tile as tile
from concourse.bass_utils import run_bass_kernel_spmd

BF = ml_dtypes.bfloat16
P = 128
N_FULL, F_FULL, NC = 4096, 512, 8
EPS = 0.5
LAM1 = 1.0
LAM2 = 1.0
LAM3 = 0.5
AF = mybir.ActivationFunctionType
ALU = mybir.AluOpType


def split_waits(nc, maxw=1):
    """This container's walrus build allows only `maxw` sync-wait commands per
    instruction. Split excess waits onto InstNoOps inserted just before, on the
    same engine (the engine executes the waits in order, so semantics are
    unchanged)."""
    for bb in nc.main_func.blocks:
        i = 0
        while i < len(bb.instructions):
            ins = bb.instructions[i]
            si = ins.sync_info
            if si and si.on_wait and len(si.on_wait) > maxw:
                waits = list(si.on_wait)
                extra, keep = waits[:-maxw], waits[-maxw:]
                si.on_wait = keep
                pos = i
                for j in range(0, len(extra), maxw):
                    nop = mybir.InstNoOp(name=f"wsplit_{ins.name}_{j}", ins=[], outs=[])
                    nop.engine = ins.engine
                    nop.sync_info = mybir.SyncInfo(on_wait=extra[j:j + maxw], on_update=[])
                    nc.register_instruction(nop)
                    bb.instructions.insert(pos, nop)
                    pos += 1
                    i += 1
            i += 1


def build(n=N_FULL, f=F_FULL, ncores=NC):
    B = n // ncores         # rows per core
    MT = B // P             # row tiles per block
    KF = f // P             # f chunks
    NKC = n // P            # node chunks
    SS = MT                 # node chunks per streamed super-strip (= rank block)
    NJ = NKC // SS
    f3 = 3 * f

    nc = bass.Bass("TRN2", target_bir_lowering=False, debug=False, num_devices=ncores)
    bf16, f32 = mybir.dt.bfloat16, mybir.dt.float32

    # ---- per-core DRAM I/O (host pre-shards / transposes / casts) ----
    dyT = nc.dram_tensor("yT", [f, B], bf16, kind="ExternalInput").ap()
    dfT = nc.dram_tensor("featT", [f, B], bf16, kind="ExternalInput").ap()
    dfeatblk = nc.dram_tensor("featblk", [B, f], bf16, kind="ExternalInput").ap()
    dfeatfull = nc.dram_tensor("featfull", [n, f], bf16, kind="ExternalInput").ap()
    dadjT = nc.dram_tensor("adjT", [n, B], bf16, kind="ExternalInput").ap()
    dadj1T = nc.dram_tensor("adj1T", [n, B], bf16, kind="ExternalInput").ap()
    dwy = nc.dram_tensor("wy", [f, f], bf16, kind="ExternalInput").ap()
    dWS = nc.dram_tensor("WS", [f, f], bf16, kind="ExternalInput").ap()
    dw2T = nc.dram_tensor("w2T", [f3, f], bf16, kind="ExternalInput").ap()
    dw2b = nc.dram_tensor("w2b", [1, f], bf16, kind="ExternalInput").ap()
    dident = nc.dram_tensor("ident", [P, P], bf16, kind="ExternalInput").ap()
    dout = nc.dram_tensor("out", [B, f], f32, kind="ExternalOutput").ap()

    with tile.TileContext(nc) as tc:
        with (
            tc.tile_pool(name="const", bufs=1) as cp,
            tc.tile_pool(name="fac", bufs=1) as fp,
            tc.tile_pool(name="stage", bufs=1) as st,
            tc.tile_pool(name="work", bufs=2) as wk,
            tc.tile_pool(name="strips", bufs=3) as sp,
            tc.tile_pool(name="ps", bufs=8, space="PSUM") as ps,
            tc.tile_pool(name="dram", bufs=1, space="DRAM") as dram,
        ):
            # ---------------- constant loads ----------------
            def load_chunked(dst_name, src, nch, width, dt=bf16, pool=cp):
                t = pool.tile([P, nch, width], dt, name=dst_name, tag=dst_name)
                nc.sync.dma_start(t[:], src.rearrange("(k p) c -> p k c", p=P))
                return t

            wy_sb = load_chunked("wy_sb", dwy, KF, f)
            WS_sb = load_chunked("WS_sb", dWS, KF, f)
            yT0 = load_chunked("yT0", dyT, KF, B)
            fT0 = load_chunked("fT0", dfT, KF, B)
            featblk_sb = load_chunked("featblk_sb", dfeatblk, MT, f)
            ident_sb = cp.tile([P, P], bf16, name="ident_sb", tag="ident_sb")
            nc.sync.dma_start(ident_sb[:], dident[:])
            w2b_sb = cp.tile([1, f], bf16, name="w2b_sb", tag="w2b_sb")
            nc.sync.dma_start(w2b_sb[:], dw2b[:])
            ones_sb = cp.tile([1, P], bf16, name="ones_sb", tag="ones_sb")
            nc.vector.memset(ones_sb[:], 1.0)

            # AllGather buffers for the layer boundary (z1, z2 blocks)
            ag_in = dram.tile([2 * B, f], bf16, name="ag_in")
            ag_out = dram.tile([2 * n, f], bf16, name="ag_out", addr_space="Shared")
            # ag_out rows: [core c][z][kc within block][p]
            ag_view = ag_out[:].rearrange(
                "(c z kc p) col -> z p c kc col", c=ncores, z=2, p=P
            )

            # ---------------- helpers ----------------
            def mm_block(dst, lhsT, rhs_sb, out_dt=bf16, eng="v"):
                """dst[128, MT_or_KF, width] = lhsT.T @ rhs ; lhsT/rhs chunked [128,KF,*]."""
                nmt = dst.shape[1]
                for mt in range(nmt):
                    psum = ps.tile([P, rhs_sb.shape[2]], f32, name="psum", tag="psum")
                    for kc in range(KF):
                        nc.tensor.matmul(
                            psum[:],
                            lhsT[:, kc, mt * P:(mt + 1) * P],
                            rhs_sb[:, kc, :],
                            start=(kc == 0),
                            stop=(kc == KF - 1),
                        )
                    if eng == "v":
                        nc.vector.tensor_copy(dst[:, mt, :], psum[:])
                    else:
                        nc.scalar.activation(dst[:, mt, :], psum[:], AF.Copy)

            def layer(xTs, temps_of, S_groups, zsrc_views, outs_cfg, lnum):
                """One DGNN layer.
                xTs: dict name -> xT tile ([128,KF,B], lhsT blocks of y/z1/z2)
                temps_of: list of (temp_name, x_name) for t = x @ WS
                S_groups: list of groups of (x_name, temp_name, coef); one
                    AllReduce per group, kicked right after its partials
                zsrc_views: list of (adjT dram, z dram view or None->featfull) for the 2 adj mms
                outs_cfg: dict describing output handling (differs l0/l1)
                """
                names = list(xTs.keys())
                # factors, normal layout: xw = x @ wy ; t = x @ WS
                # (xwT is NOT needed for the S partials -> computed after the
                # AllReduces are kicked, so the collectives start ASAP)
                xw = {}
                tt = {}
                xwT = {}
                for x in names:
                    xw[x] = fp.tile([P, MT, f], bf16, name=f"xw_{x}", tag=f"xw_{x}")
                    mm_block(xw[x], xTs[x], wy_sb)

                # S partials with coefficient folded in (bf16 AllReduce), split
                # into groups: the z-path group (t24/t34) is reduced first so
                # its sigmoid matmuls unblock earlier.
                temps_x = dict((t, x) for t, x in temps_of)
                S_lookup = {}
                for gi, group in enumerate(S_groups):
                    for tname in sorted({t for (_, t, _) in group}):
                        if tname not in tt:
                            tt[tname] = fp.tile([P, MT, f], bf16,
                                                name=f"tt_{tname}", tag=f"tt_{tname}")
                            mm_block(tt[tname], xTs[temps_x[tname]], WS_sb)
                    nS = len(group)
                    S_stage = st.tile([P, nS * KF, f], bf16,
                                      name=f"S_stage_{gi}", tag=f"S_stage_{gi}")
                    for si, (x, tname, coef) in enumerate(group):
                        for mt in range(KF):  # m dim of S is f
                            psum = ps.tile([P, f], f32, name="psum", tag="psum")
                            for kc in range(MT):  # contraction over rows of block
                                nc.tensor.matmul(
                                    psum[:],
                                    xw[x][:, kc, mt * P:(mt + 1) * P],
                                    tt[tname][:, kc, :],
                                    start=(kc == 0),
                                    stop=(kc == MT - 1),
                                )
                            nc.scalar.activation(
                                S_stage[:, si * KF + mt, :], psum[:], AF.Copy,
                                scale=float(coef)
                            )
                    ar_in = dram.tile([nS * f, f], bf16, name=f"ar_in_{lnum}_{gi}")
                    ar_out = dram.tile([nS * f, f], bf16, name=f"ar_out_{lnum}_{gi}",
                                       addr_space="Shared")
                    nc.gpsimd.dma_start(ar_in[:].rearrange("(k p) c -> p k c", p=P), S_stage[:])
                    nc.gpsimd.collective_compute(
                        "AllReduce", ALU.add,
                        replica_groups=[list(range(ncores))],
                        ins=[ar_in[:].opt()], outs=[ar_out[:].opt()],
                    )
                    S_sb = st.tile([P, nS * KF, f], bf16,
                                   name=f"S_sb_{gi}", tag=f"S_stage_{gi}")
                    nc.gpsimd.dma_start(S_sb[:], ar_out[:].rearrange("(k p) c -> p k c", p=P))
                    for si, (x, t, _) in enumerate(group):
                        S_lookup[(x, t)] = (S_sb, si)

                # xwT = xw.T via PE transposes (cheaper than matmul, runs during AR)
                for x in names:
                    xwT[x] = fp.tile([P, KF, B], bf16, name=f"xwT_{x}", tag=f"xwT_{x}")
                    for jf in range(KF):
                        pst = ps.tile([P, B], bf16, name="psum_tr", tag="psum")
                        for it in range(MT):
                            nc.tensor.transpose(
                                pst[:, it * P:(it + 1) * P],
                                xw[x][:, it, jf * P:(jf + 1) * P],
                                ident_sb[:],
                            )
                        nc.vector.tensor_copy(xwT[x][:, jf, :], pst[:])

                def sig_mm(tname, transposed, dst, dst_sl, out_dt):
                    """sigmoid(sum_x xw_x @ S_{x,t}) into dst[:, dst_sl, :].
                    transposed=False: out[r, fo'] via lhsT=xwT ; True: out[fo', r] via lhsT=S."""
                    pairs = [(x, t) for g in S_groups for (x, t, _) in g if t == tname]
                    nmt = MT if not transposed else KF
                    res = []
                    for mt in range(nmt):
                        psum = ps.tile([P, B if transposed else f], f32, name="psum", tag="psum")
                        k = 0
                        nmm = len(pairs) * KF
                        for (x, t) in pairs:
                            sb, si = S_lookup[(x, t)]
                            for kc in range(KF):
                                if not transposed:
                                    lhsT = xwT[x][:, kc, mt * P:(mt + 1) * P]
                                    rhs = sb[:, si * KF + kc, :]
                                else:
                                    lhsT = sb[:, si * KF + kc, mt * P:(mt + 1) * P]
                                    rhs = xwT[x][:, kc, :]
                                nc.tensor.matmul(psum[:], lhsT, rhs,
                                                 start=(k == 0), stop=(k == nmm - 1))
                                k += 1
                        res.append(psum)
                    return res  # list of PSUM tiles (pre-sigmoid)

                # adj matmuls, normal layout: out[r, f] ; lhsT = adjT strips, rhs = z strips
                def adj_mm(dadj, z_strip_of):
                    psums = [ps.tile([P, f], f32, name="psum_adj", tag="psum")
                             for _ in range(MT)]
                    adj_re = dadj.rearrange("(k p) r -> p k r", p=P)
                    for j in range(NJ):
                        astrip = sp.tile([P, SS, B], bf16, name="astrip", tag="astrip")
                        nc.sync.dma_start(astrip[:], adj_re[:, j * SS:(j + 1) * SS, :])
                        zstrip = sp.tile([P, SS, f], bf16, name="zstrip", tag="zstrip")
                        nc.sync.dma_start(zstrip[:], z_strip_of(j))
                        for q in range(SS):
                            kc = j * SS + q
                            for mt in range(MT):
                                nc.tensor.matmul(
                                    psums[mt][:],
                                    astrip[:, q, mt * P:(mt + 1) * P],
                                    zstrip[:, q, :],
                                    start=(kc == 0),
                                    stop=(kc == NKC - 1),
                                )
                    return psums

                return xw, xwT, tt, sig_mm, adj_mm

            # ======================= LAYER 0 (z1 = z2 = feat) =======================
            xTs0 = {"y": yT0, "z1": fT0}
            temps0 = [("t12", "y"), ("t24", "z1")]
            S0 = [[("y", "t24", -1.0), ("z1", "t24", 1.0)],
                  [("y", "t12", 1.0), ("z1", "t12", -1.0)]]
            featfull_view = dfeatfull.rearrange("(k p) c -> p k c", p=P)
            xw0, xwT0, tt0, sig_mm0, adj_mm0 = layer(xTs0, temps0, S0, None, None, 0)

            # adj matmuls first: they do not depend on the AllReduce, so the PE
            # works on them while the collective is in flight. Copy PSUM->SBUF
            # to free banks for the sigmoid matmuls.
            adj_out0 = []
            for ai, dadj in enumerate((dadjT, dadj1T)):
                psums = adj_mm0(dadj, lambda j: featfull_view[:, j * SS:(j + 1) * SS, :])
                ao = st.tile([P, MT, f], bf16, name=f"adj_out{ai}", tag=f"adj_out{ai}")
                for mt in range(MT):
                    nc.vector.tensor_copy(ao[:, mt, :], psums[mt][:])
                adj_out0.append(ao)

            # sig2 (normal layout), shared by z1_n and z2_n at layer 0
            pre2 = sig_mm0("t24", False, None, None, None)
            sig2_0 = st.tile([P, MT, f], bf16, name="sig2_0", tag="sig2_0")
            for mt in range(MT):
                nc.scalar.activation(sig2_0[:, mt, :], pre2[mt][:], AF.Sigmoid)

            # z1_n = adj@feat - 0.25*sig2 ; z2_n = adj1@feat - 0.25*sig2  (normal layout)
            z1s = st.tile([P, MT, f], bf16, name="z1s", tag="z1s")
            z2s = st.tile([P, MT, f], bf16, name="z2s", tag="z2s")
            for (ao, dst, cf) in [(adj_out0[0], z1s, EPS * LAM3 / LAM1),
                                  (adj_out0[1], z2s, (1.0 - EPS) * LAM3 / LAM2)]:
                for mt in range(MT):
                    nc.vector.scalar_tensor_tensor(
                        dst[:, mt, :], sig2_0[:, mt, :], -cf, ao[:, mt, :],
                        op0=ALU.mult, op1=ALU.add,
                    )

            # AllGather z1/z2 blocks -> full (normal layout) for layer-1 adj matmuls
            nc.gpsimd.dma_start(ag_in[0:B, :].rearrange("(k p) c -> p k c", p=P), z1s[:])
            nc.gpsimd.dma_start(ag_in[B:2 * B, :].rearrange("(k p) c -> p k c", p=P), z2s[:])
            nc.gpsimd.collective_compute(
                "AllGather", ALU.bypass,
                replica_groups=[list(range(ncores))],
                ins=[ag_in[:].opt()], outs=[ag_out[:].opt()],
            )

            # y_nT = featT - 0.5*sigmoid(pre1T)   (T layout -> next-layer lhsT directly)
            yT2 = st.tile([P, KF, B], bf16, name="yT2", tag="yT2")
            pre1T = sig_mm0("t12", True, None, None, None)
            for mt in range(KF):
                sg = wk.tile([P, B], bf16, name="sg_t", tag="sg_t")
                nc.scalar.activation(sg[:], pre1T[mt][:], AF.Sigmoid)
                nc.vector.scalar_tensor_tensor(
                    yT2[:, mt, :], sg[:], -LAM3, fT0[:, mt, :], op0=ALU.mult, op1=ALU.add
                )

            # local transposes: z1T2/z2T2 = (z1_n block).T for layer-1 factor lhsT
            z1T2 = st.tile([P, KF, B], bf16, name="z1T2", tag="z1T2")
            z2T2 = st.tile([P, KF, B], bf16, name="z2T2", tag="z2T2")
            for (src, dst) in [(z1s, z1T2), (z2s, z2T2)]:
                for jf in range(KF):
                    psum = ps.tile([P, B], bf16, name="psum_tr", tag="psum")
                    for it in range(MT):
                        nc.tensor.transpose(
                            psum[:, it * P:(it + 1) * P],
                            src[:, it, jf * P:(jf + 1) * P],
                            ident_sb[:],
                        )
                    nc.vector.tensor_copy(dst[:, jf, :], psum[:])

            # ======================= LAYER 1 =======================
            xTs1 = {"y": yT2, "z1": z1T2, "z2": z2T2}
            temps1 = [("t12", "y"), ("t24", "z1"), ("t34", "z2")]
            S1 = [[("y", "t24", -1.0), ("z1", "t24", EPS), ("z2", "t24", (1 - EPS)),
                   ("y", "t34", -1.0), ("z1", "t34", EPS), ("z2", "t34", (1 - EPS)),
                   ("y", "t12", 1.0), ("z1", "t12", -EPS), ("z2", "t12", -(1 - EPS))]]
            xw1, xwT1, tt1, sig_mm1, adj_mm1 = layer(xTs1, temps1, S1, None, None, 1)

            # final y/z1/z2 blocks in fp32 (normal layout)
            fin = {}
            for name in ("y", "z1", "z2"):
                fin[name] = st.tile([P, MT, f], bf16, name=f"fin_{name}", tag=f"fin_{name}")

            # adj matmuls first (depend on the AllGather, not the AllReduce)
            adj_out1 = []
            for ai, (dadj, zi) in enumerate(((dadjT, 0), (dadj1T, 1))):
                psums = adj_mm1(dadj, lambda j, z=zi: ag_view[z][:, j, :, :])
                ao = st.tile([P, MT, f], bf16, name=f"adj_out{ai}", tag=f"adj_out{ai}")
                for mt in range(MT):
                    nc.vector.tensor_copy(ao[:, mt, :], psums[mt][:])
                adj_out1.append(ao)

            for (tname, ao, dst, cf) in [
                ("t24", adj_out1[0], fin["z1"], EPS * LAM3 / LAM1),
                ("t34", adj_out1[1], fin["z2"], (1 - EPS) * LAM3 / LAM2),
            ]:
                pre = sig_mm1(tname, False, None, None, None)
                for mt in range(MT):
                    sg = wk.tile([P, f], f32, name="sg_f", tag="sg_f")
                    nc.scalar.activation(sg[:], pre[mt][:], AF.Sigmoid)
                    nc.vector.scalar_tensor_tensor(
                        dst[:, mt, :], sg[:], -cf, ao[:, mt, :],
                        op0=ALU.mult, op1=ALU.add,
                    )

            pre1 = sig_mm1("t12", False, None, None, None)
            for mt in range(MT):
                sg = wk.tile([P, f], f32, name="sg_f", tag="sg_f")
                nc.scalar.activation(sg[:], pre1[mt][:], AF.Sigmoid)
                nc.vector.scalar_tensor_tensor(
                    fin["y"][:, mt, :], sg[:], -LAM3, featblk_sb[:, mt, :],
                    op0=ALU.mult, op1=ALU.add,
                )

            # ======================= FINAL: l2norm, logits, log_softmax =======================
            pT = st.tile([P, 3 * KF, B], bf16, name="pT", tag="pT")
            for xi, name in [(1, "z1"), (2, "z2"), (0, "y")]:
                x = fin[name]
                normed = wk.tile([P, MT, f], bf16, name="normed", tag="normed", bufs=1)
                for rc in range(MT):
                    sq = wk.tile([P, f], f32, name="sq", tag="lg")
                    ssq = wk.tile([P, 1], f32, name="ssq", tag="ssq")
                    nc.scalar.activation(sq[:], x[:, rc, :], AF.Square, accum_out=ssq[:])
                    nrm = wk.tile([P, 1], f32, name="nrm", tag="nrm")
                    nc.scalar.activation(nrm[:], ssq[:], AF.Sqrt)
                    nc.vector.tensor_scalar_max(nrm[:], nrm[:], 1e-12)
                    rn = wk.tile([P, 1], f32, name="rn", tag="rn")
                    nc.vector.reciprocal(rn[:], nrm[:])
                    nc.scalar.activation(normed[:, rc, :], x[:, rc, :], AF.Copy, scale=rn[:])
                # transpose normed [B, f] -> pT rows [f, B]
                for jf in range(KF):
                    psum = ps.tile([P, B], bf16, name="psum_tr", tag="psum")
                    for it in range(MT):
                        nc.tensor.transpose(
                            psum[:, it * P:(it + 1) * P],
                            normed[:, it, jf * P:(jf + 1) * P],
                            ident_sb[:],
                        )
                    nc.vector.tensor_copy(pT[:, xi * KF + jf, :], psum[:])

            w2T_re = dw2T.rearrange("(k p) c -> p k c", p=P)
            w2strips = []
            for j in range(3 * KF // SS):
                w2s = sp.tile([P, SS, f], bf16, name="w2s", tag="astrip")
                nc.sync.dma_start(w2s[:], w2T_re[:, j * SS:(j + 1) * SS, :])
                w2strips.append(w2s)
            for mt in range(MT):
                psum = ps.tile([P, f], f32, name="psum", tag="psum")
                for kc in range(3 * KF):
                    nc.tensor.matmul(psum[:], pT[:, kc, mt * P:(mt + 1) * P],
                                     w2strips[kc // SS][:, kc % SS, :],
                                     start=(kc == 0), stop=False)
                nc.tensor.matmul(psum[:], ones_sb[:], w2b_sb[:], start=False, stop=True)
                # log_softmax (fp32)
                L = wk.tile([P, f], f32, name="lg", tag="lg")
                nc.vector.tensor_copy(L[:], psum[:])
                nmx = wk.tile([P, 1], f32, name="nmx", tag="nmx")
                nc.vector.tensor_reduce(out=nmx[:], in_=L[:], op=ALU.max,
                                        axis=mybir.AxisListType.X, negate=True)
                ex = wk.tile([P, f], f32, name="ex", tag="ex")
                se = wk.tile([P, 1], f32, name="se", tag="se")
                nc.scalar.activation(ex[:], L[:], AF.Exp, bias=nmx[:], accum_out=se[:])
                lse = wk.tile([P, 1], f32, name="lse", tag="lse")
                nc.scalar.activation(lse[:], se[:], AF.Ln)
                b2 = wk.tile([P, 1], f32, name="b2", tag="b2")
                nc.vector.tensor_sub(b2[:], nmx[:], lse[:])
                lo = wk.tile([P, f], f32, name="lo", tag="lo")
                nc.scalar.activation(lo[:], L[:], AF.Identity, bias=b2[:])
                nc.sync.dma_start(dout[mt * P:(mt + 1) * P, :], lo[:])

    split_waits(nc, maxw=1)
    return nc


def prep_inputs(feat, adj, adj1, y, wy, w2_w, w2_b, n=N_FULL, ncores=NC):
    """Host-side sharding / transposition / casting. Returns in_maps."""
    B = n // ncores
    wy64 = np.asarray(wy, np.float64)
    WS = wy64 @ wy64.T + wy64.T @ wy64
    wy_bf = np.asarray(wy, BF)
    WS_bf = WS.astype(np.float32).astype(BF)
    w2T_bf = np.ascontiguousarray(np.asarray(w2_w, np.float32).T).astype(BF)
    w2b_bf = np.asarray(w2_b, np.float32).reshape(1, -1).astype(BF)
    ident = np.eye(P, dtype=np.float32).astype(BF)
    featfull_bf = np.asarray(feat, np.float32).astype(BF)
    in_maps = []
    for c in range(ncores):
        r0, r1 = c * B, (c + 1) * B
        in_maps.append({
            "yT": np.ascontiguousarray(np.asarray(y[r0:r1], np.float32).T).astype(BF),
            "featT": np.ascontiguousarray(np.asarray(feat[r0:r1], np.float32).T).astype(BF),
            "featblk": np.asarray(feat[r0:r1], np.float32).astype(BF),
            "featfull": featfull_bf,
            "adjT": np.ascontiguousarray(np.asarray(adj[r0:r1], np.float32).T).astype(BF),
            "adj1T": np.ascontiguousarray(np.asarray(adj1[r0:r1], np.float32).T).astype(BF),
            "wy": wy_bf,
            "WS": WS_bf,
            "w2T": w2T_bf,
            "w2b": w2b_bf,
            "ident": ident,
        })
    return in_maps


_CACHED_NC = None


def kernel(feat, adj, adj1, y, wy, w2_w, w2_b):
    global _CACHED_NC
    if _CACHED_NC is None:
        _CACHED_NC = build()
    in_maps = prep_inputs(feat, adj, adj1, y, wy, w2_w, w2_b)
    res = run_bass_kernel_spmd(_CACHED_NC, in_maps, core_ids=list(range(NC)))
    return np.concatenate([res.results[c]["out"] for c in range(NC)], axis=0)


# revision 17
# speedup vs baseline: 1.0599x; 1.0599x over previous
"""DGNN message-passing kernel for 8 TRN2 NeuronCores.

Row-shards the n dimension across 8 cores. Algebraic restructuring: the n x n
Gram matrices (temp11 = yw@yw.T - eps*z1w@z1w.T - ...) are never materialized.
Instead, for each sigmoid argument:

    (temp11 @ t)[rows_c, :] = sum_x coef_x * xw_x[rows_c, :] @ S_{x,t}
    with S_{x,t} = xw_x.T @ t = sum_c xw_x[rows_c].T @ t[rows_c]   (f x f)

The S matrices are computed as per-core block partials and AllReduced (f x f
each). The only O(n^2) matmuls left are adj @ z1 / adj1 @ z2, block-rows per
core. Between layers, z1/z2 row-blocks are AllGathered (y is only ever needed
block-wise). All heavy matmuls run in bf16 (fp32 PSUM accumulation); the
sigmoids saturate, final rel err vs fp64 reference ~1e-5.
"""

import numpy as np
import ml_dtypes

import concourse.bass as bass
import concourse.mybir as mybir
import concourse.tile as tile
from concourse.bass_utils import run_bass_kernel_spmd

BF = ml_dtypes.bfloat16
P = 128
N_FULL, F_FULL, NC = 4096, 512, 8
EPS = 0.5
LAM1 = 1.0
LAM2 = 1.0
LAM3 = 0.5
AF = mybir.ActivationFunctionType
ALU = mybir.AluOpType


def split_waits(nc, maxw=1):
    """This container's walrus build allows only `maxw` sync-wait commands per
    instruction. Split excess waits onto InstNoOps inserted just before, on the
    same engine (the engine executes the waits in order, so semantics are
    unchanged)."""
    for bb in nc.main_func.blocks:
        i = 0
        while i < len(bb.instructions):
            ins = bb.instructions[i]
            si = ins.sync_info
            if si and si.on_wait and len(si.on_wait) > maxw:
                waits = list(si.on_wait)
                extra, keep = waits[:-maxw], waits[-maxw:]
                si.on_wait = keep
                pos = i
                for j in range(0, len(extra), maxw):
                    nop = mybir.InstNoOp(name=f"wsplit_{ins.name}_{j}", ins=[], outs=[])
                    nop.engine = ins.engine
                    nop.sync_info = mybir.SyncInfo(on_wait=extra[j:j + maxw], on_update=[])
                    nc.register_instruction(nop)
                    bb.instructions.insert(pos, nop)
                    pos += 1
                    i += 1
            i += 1


def build(n=N_FULL, f=F_FULL, ncores=NC):
    B = n // ncores         # rows per core
    MT = B // P             # row tiles per block
    KF = f // P             # f chunks
    NKC = n // P            # node chunks
    SS = MT                 # node chunks per streamed super-strip (= rank block)
    NJ = NKC // SS
    f3 = 3 * f

    nc = bass.Bass("TRN2", target_bir_lowering=False, debug=False, num_devices=ncores)
    bf16, f32 = mybir.dt.bfloat16, mybir.dt.float32

    # ---- per-core DRAM I/O (host pre-shards / transposes / casts) ----
    dyT = nc.dram_tensor("yT", [f, B], bf16, kind="ExternalInput").ap()
    dfT = nc.dram_tensor("featT", [f, B], bf16, kind="ExternalInput").ap()
    dfeatblk = nc.dram_tensor("featblk", [B, f], bf16, kind="ExternalInput").ap()
    dfeatfull = nc.dram_tensor("featfull", [n, f], bf16, kind="ExternalInput").ap()
    dadjT = nc.dram_tensor("adjT", [n, B], bf16, kind="ExternalInput").ap()
    dadj1T = nc.dram_tensor("adj1T", [n, B], bf16, kind="ExternalInput").ap()
    dwy = nc.dram_tensor("wy", [f, f], bf16, kind="ExternalInput").ap()
    dWS = nc.dram_tensor("WS", [f, f], bf16, kind="ExternalInput").ap()
    dw2T = nc.dram_tensor("w2T", [f3, f], bf16, kind="ExternalInput").ap()
    dw2b = nc.dram_tensor("w2b", [1, f], bf16, kind="ExternalInput").ap()
    dident = nc.dram_tensor("ident", [P, P], bf16, kind="ExternalInput").ap()
    dout = nc.dram_tensor("out", [B, f], f32, kind="ExternalOutput").ap()

    with tile.TileContext(nc) as tc:
        with (
            tc.tile_pool(name="const", bufs=1) as cp,
            tc.tile_pool(name="fac", bufs=1) as fp,
            tc.tile_pool(name="stage", bufs=1) as st,
            tc.tile_pool(name="work", bufs=2) as wk,
            tc.tile_pool(name="strips", bufs=3) as sp,
            tc.tile_pool(name="ps", bufs=8, space="PSUM") as ps,
            tc.tile_pool(name="dram", bufs=1, space="DRAM") as dram,
        ):
            # ---------------- constant loads ----------------
            def load_chunked(dst_name, src, nch, width, dt=bf16, pool=cp):
                t = pool.tile([P, nch, width], dt, name=dst_name, tag=dst_name)
                nc.sync.dma_start(t[:], src.rearrange("(k p) c -> p k c", p=P))
                return t

            wy_sb = load_chunked("wy_sb", dwy, KF, f)
            WS_sb = load_chunked("WS_sb", dWS, KF, f)
            yT0 = load_chunked("yT0", dyT, KF, B)
            fT0 = load_chunked("fT0", dfT, KF, B)
            featblk_sb = load_chunked("featblk_sb", dfeatblk, MT, f)
            ident_sb = cp.tile([P, P], bf16, name="ident_sb", tag="ident_sb")
            nc.sync.dma_start(ident_sb[:], dident[:])
            w2b_sb = cp.tile([1, f], bf16, name="w2b_sb", tag="w2b_sb")
            nc.sync.dma_start(w2b_sb[:], dw2b[:])
            ones_sb = cp.tile([1, P], bf16, name="ones_sb", tag="ones_sb")
            nc.vector.memset(ones_sb[:], 1.0)

            # AllGather buffers for the layer boundary (z1, z2 blocks)
            ag_in = dram.tile([2 * B, f], bf16, name="ag_in")
            ag_out = dram.tile([2 * n, f], bf16, name="ag_out", addr_space="Shared")
            # ag_out rows: [core c][z][kc within block][p]
            ag_view = ag_out[:].rearrange(
                "(c z kc p) col -> z p c kc col", c=ncores, z=2, p=P
            )

            # ---------------- helpers ----------------
            def mm_block(dst, lhsT, rhs_sb, out_dt=bf16, eng="v"):
                """dst[128, MT_or_KF, width] = lhsT.T @ rhs ; lhsT/rhs chunked [128,KF,*]."""
                nmt = dst.shape[1]
                for mt in range(nmt):
                    psum = ps.tile([P, rhs_sb.shape[2]], f32, name="psum", tag="psum")
                    for kc in range(KF):
                        nc.tensor.matmul(
                            psum[:],
                            lhsT[:, kc, mt * P:(mt + 1) * P],
                            rhs_sb[:, kc, :],
                            start=(kc == 0),
                            stop=(kc == KF - 1),
                        )
                    if eng == "v":
                        nc.vector.tensor_copy(dst[:, mt, :], psum[:])
                    else:
                        nc.scalar.activation(dst[:, mt, :], psum[:], AF.Copy)

            def layer(xTs, temps_of, S_groups, zsrc_views, outs_cfg, lnum):
                """One DGNN layer.
                xTs: dict name -> xT tile ([128,KF,B], lhsT blocks of y/z1/z2)
                temps_of: list of (temp_name, x_name) for t = x @ WS
                S_groups: list of groups of (x_name, temp_name, coef); one
                    AllReduce per group, kicked right after its partials
                zsrc_views: list of (adjT dram, z dram view or None->featfull) for the 2 adj mms
                outs_cfg: dict describing output handling (differs l0/l1)
                """
                names = list(xTs.keys())
                # factors, normal layout: xw = x @ wy ; t = x @ WS
                # (xwT is NOT needed for the S partials -> computed after the
                # AllReduces are kicked, so the collectives start ASAP)
                xw = {}
                tt = {}
                xwT = {}
                for x in names:
                    xw[x] = fp.tile([P, MT, f], bf16, name=f"xw_{x}", tag=f"xw_{x}")
                    mm_block(xw[x], xTs[x], wy_sb)

                # S partials with coefficient folded in (bf16 AllReduce), split
                # into groups: the z-path group (t24/t34) is reduced first so
                # its sigmoid matmuls unblock earlier.
                temps_x = dict((t, x) for t, x in temps_of)
                S_lookup = {}
                for gi, group in enumerate(S_groups):
                    for tname in sorted({t for (_, t, _) in group}):
                        if tname not in tt:
                            tt[tname] = fp.tile([P, MT, f], bf16,
                                                name=f"tt_{tname}", tag=f"tt_{tname}")
                            mm_block(tt[tname], xTs[temps_x[tname]], WS_sb)
                    nS = len(group)
                    S_stage = st.tile([P, nS * KF, f], bf16,
                                      name=f"S_stage_{gi}", tag=f"S_stage_{gi}")
                    for si, (x, tname, coef) in enumerate(group):
                        for mt in range(KF):  # m dim of S is f
                            psum = ps.tile([P, f], f32, name="psum", tag="psum")
                            for kc in range(MT):  # contraction over rows of block
                                nc.tensor.matmul(
                                    psum[:],
                                    xw[x][:, kc, mt * P:(mt + 1) * P],
                                    tt[tname][:, kc, :],
                                    start=(kc == 0),
                                    stop=(kc == MT - 1),
                                )
                            nc.scalar.activation(
                                S_stage[:, si * KF + mt, :], psum[:], AF.Copy,
                                scale=float(coef)
                            )
                    ar_in = dram.tile([nS * f, f], bf16, name=f"ar_in_{lnum}_{gi}")
                    ar_out = dram.tile([nS * f, f], bf16, name=f"ar_out_{lnum}_{gi}",
                                       addr_space="Shared")
                    nc.gpsimd.dma_start(ar_in[:].rearrange("(k p) c -> p k c", p=P), S_stage[:])
                    nc.gpsimd.collective_compute(
                        "AllReduce", ALU.add,
                        replica_groups=[list(range(ncores))],
                        ins=[ar_in[:].opt()], outs=[ar_out[:].opt()],
                    )
                    S_sb = st.tile([P, nS * KF, f], bf16,
                                   name=f"S_sb_{gi}", tag=f"S_stage_{gi}")
                    nc.gpsimd.dma_start(S_sb[:], ar_out[:].rearrange("(k p) c -> p k c", p=P))
                    for si, (x, t, _) in enumerate(group):
                        S_lookup[(x, t)] = (S_sb, si)

                # xwT = xw.T via PE transposes (cheaper than matmul, runs during AR)
                for x in names:
                    xwT[x] = fp.tile([P, KF, B], bf16, name=f"xwT_{x}", tag=f"xwT_{x}")
                    for jf in range(KF):
                        pst = ps.tile([P, B], bf16, name="psum_tr", tag="psum")
                        for it in range(MT):
                            nc.tensor.transpose(
                                pst[:, it * P:(it + 1) * P],
                                xw[x][:, it, jf * P:(jf + 1) * P],
                                ident_sb[:],
                            )
                        nc.vector.tensor_copy(xwT[x][:, jf, :], pst[:])

                def sig_mm(tname, transposed, dst, dst_sl, out_dt):
                    """sigmoid(sum_x xw_x @ S_{x,t}) into dst[:, dst_sl, :].
                    transposed=False: out[r, fo'] via lhsT=xwT ; True: out[fo', r] via lhsT=S."""
                    pairs = [(x, t) for g in S_groups for (x, t, _) in g if t == tname]
                    nmt = MT if not transposed else KF
                    res = []
                    for mt in range(nmt):
                        psum = ps.tile([P, B if transposed else f], f32, name="psum", tag="psum")
                        k = 0
                        nmm = len(pairs) * KF
                        for (x, t) in pairs:
                            sb, si = S_lookup[(x, t)]
                            for kc in range(KF):
                                if not transposed:
                                    lhsT = xwT[x][:, kc, mt * P:(mt + 1) * P]
                                    rhs = sb[:, si * KF + kc, :]
                                else:
                                    lhsT = sb[:, si * KF + kc, mt * P:(mt + 1) * P]
                                    rhs = xwT[x][:, kc, :]
                                nc.tensor.matmul(psum[:], lhsT, rhs,
                                                 start=(k == 0), stop=(k == nmm - 1))
                                k += 1
                        res.append(psum)
                    return res  # list of PSUM tiles (pre-sigmoid)

                # adj matmuls, normal layout: out[r, f] ; lhsT = adjT strips, rhs = z strips
                def adj_mm(dadj, z_strip_of):
                    psums = [ps.tile([P, f], f32, name="psum_adj", tag="psum")
                             for _ in range(MT)]
                    adj_re = dadj.rearrange("(k p) r -> p k r", p=P)
                    for j in range(NJ):
                        astrip = sp.tile([P, SS, B], bf16, name="astrip", tag="astrip")
                        nc.sync.dma_start(astrip[:], adj_re[:, j * SS:(j + 1) * SS, :])
                        zstrip = sp.tile([P, SS, f], bf16, name="zstrip", tag="zstrip")
                        nc.sync.dma_start(zstrip[:], z_strip_of(j))
                        for q in range(SS):
                            kc = j * SS + q
                            for mt in range(MT):
                                nc.tensor.matmul(
                                    psums[mt][:],
                                    astrip[:, q, mt * P:(mt + 1) * P],
                                    zstrip[:, q, :],
                                    start=(kc == 0),
                                    stop=(kc == NKC - 1),
                                )
                    return psums

                return xw, xwT, tt, sig_mm, adj_mm

            # ======================= LAYER 0 (z1 = z2 = feat) =======================
            xTs0 = {"y": yT0, "z1": fT0}
            temps0 = [("t12", "y"), ("t24", "z1")]
            S0 = [[("y", "t24", -1.0), ("z1", "t24", 1.0)],
                  [("y", "t12", 1.0), ("z1", "t12", -1.0)]]
            featfull_view = dfeatfull.rearrange("(k p) c -> p k c", p=P)
            xw0, xwT0, tt0, sig_mm0, adj_mm0 = layer(xTs0, temps0, S0, None, None, 0)

            # adj matmuls first: they do not depend on the AllReduce, so the PE
            # works on them while the collective is in flight. Copy PSUM->SBUF
            # to free banks for the sigmoid matmuls.
            adj_out0 = []
            for ai, dadj in enumerate((dadjT, dadj1T)):
                psums = adj_mm0(dadj, lambda j: featfull_view[:, j * SS:(j + 1) * SS, :])
                ao = st.tile([P, MT, f], bf16, name=f"adj_out{ai}", tag=f"adj_out{ai}")
                for mt in range(MT):
                    nc.vector.tensor_copy(ao[:, mt, :], psums[mt][:])
                adj_out0.append(ao)

            # sig2 (normal layout), shared by z1_n and z2_n at layer 0
            pre2 = sig_mm0("t24", False, None, None, None)
            sig2_0 = st.tile([P, MT, f], bf16, name="sig2_0", tag="sig2_0")
            for mt in range(MT):
                nc.scalar.activation(sig2_0[:, mt, :], pre2[mt][:], AF.Sigmoid)

            # z1_n = adj@feat - 0.25*sig2 ; z2_n = adj1@feat - 0.25*sig2  (normal layout)
            z1s = st.tile([P, MT, f], bf16, name="z1s", tag="z1s")
            z2s = st.tile([P, MT, f], bf16, name="z2s", tag="z2s")
            for (ao, dst, cf) in [(adj_out0[0], z1s, EPS * LAM3 / LAM1),
                                  (adj_out0[1], z2s, (1.0 - EPS) * LAM3 / LAM2)]:
                for mt in range(MT):
                    nc.vector.scalar_tensor_tensor(
                        dst[:, mt, :], sig2_0[:, mt, :], -cf, ao[:, mt, :],
                        op0=ALU.mult, op1=ALU.add,
                    )

            # AllGather z1/z2 blocks -> full (normal layout) for layer-1 adj matmuls
            nc.gpsimd.dma_start(ag_in[0:B, :].rearrange("(k p) c -> p k c", p=P), z1s[:])
            nc.gpsimd.dma_start(ag_in[B:2 * B, :].rearrange("(k p) c -> p k c", p=P), z2s[:])
            nc.gpsimd.collective_compute(
                "AllGather", ALU.bypass,
                replica_groups=[list(range(ncores))],
                ins=[ag_in[:].opt()], outs=[ag_out[:].opt()],
            )

            # y_nT = featT - 0.5*sigmoid(pre1T)   (T layout -> next-layer lhsT directly)
            yT2 = st.tile([P, KF, B], bf16, name="yT2", tag="yT2")
            pre1T = sig_mm0("t12", True, None, None, None)
            for mt in range(KF):
                sg = wk.tile([P, B], bf16, name="sg_t", tag="sg_t")
                nc.scalar.activation(sg[:], pre1T[mt][:], AF.Sigmoid)
                nc.vector.scalar_tensor_tensor(
                    yT2[:, mt, :], sg[:], -LAM3, fT0[:, mt, :], op0=ALU.mult, op1=ALU.add
                )

            # local transposes: z1T2/z2T2 = (z1_n block).T for layer-1 factor lhsT
            z1T2 = st.tile([P, KF, B], bf16, name="z1T2", tag="z1T2")
            z2T2 = st.tile([P, KF, B], bf16, name="z2T2", tag="z2T2")
            for (src, dst) in [(z1s, z1T2), (z2s, z2T2)]:
                for jf in range(KF):
                    psum = ps.tile([P, B], bf16, name="psum_tr", tag="psum")
                    for it in range(MT):
                        nc.tensor.transpose(
                            psum[:, it * P:(it + 1) * P],
                            src[:, it, jf * P:(jf + 1) * P],
                            ident_sb[:],
                        )
                    nc.vector.tensor_copy(dst[:, jf, :], psum[:])

            # ======================= LAYER 1 =======================
            xTs1 = {"y": yT2, "z1": z1T2, "z2": z2T2}
            temps1 = [("t12", "y"), ("t24", "z1"), ("t34", "z2")]
            S1 = [[("y", "t24", -1.0), ("z1", "t24", EPS), ("z2", "t24", (1 - EPS)),
                   ("y", "t34", -1.0), ("z1", "t34", EPS), ("z2", "t34", (1 - EPS))],
                  [("y", "t12", 1.0), ("z1", "t12", -EPS), ("z2", "t12", -(1 - EPS))]]
            xw1, xwT1, tt1, sig_mm1, adj_mm1 = layer(xTs1, temps1, S1, None, None, 1)

            # final y/z1/z2 blocks in fp32 (normal layout)
            fin = {}
            for name in ("y", "z1", "z2"):
                fin[name] = st.tile([P, MT, f], bf16, name=f"fin_{name}", tag=f"fin_{name}")

            # adj matmuls first (depend on the AllGather, not the AllReduce)
            adj_out1 = []
            for ai, (dadj, zi) in enumerate(((dadjT, 0), (dadj1T, 1))):
                psums = adj_mm1(dadj, lambda j, z=zi: ag_view[z][:, j, :, :])
                ao = st.tile([P, MT, f], bf16, name=f"adj_out{ai}", tag=f"adj_out{ai}")
                for mt in range(MT):
                    nc.vector.tensor_copy(ao[:, mt, :], psums[mt][:])
                adj_out1.append(ao)

            for (tname, ao, dst, cf) in [
                ("t24", adj_out1[0], fin["z1"], EPS * LAM3 / LAM1),
                ("t34", adj_out1[1], fin["z2"], (1 - EPS) * LAM3 / LAM2),
            ]:
                pre = sig_mm1(tname, False, None, None, None)
                for mt in range(MT):
                    sg = wk.tile([P, f], f32, name="sg_f", tag="sg_f")
                    nc.scalar.activation(sg[:], pre[mt][:], AF.Sigmoid)
                    nc.vector.scalar_tensor_tensor(
                        dst[:, mt, :], sg[:], -cf, ao[:, mt, :],
                        op0=ALU.mult, op1=ALU.add,
                    )

            pre1 = sig_mm1("t12", False, None, None, None)
            for mt in range(MT):
                sg = wk.tile([P, f], f32, name="sg_f", tag="sg_f")
                nc.scalar.activation(sg[:], pre1[mt][:], AF.Sigmoid)
                nc.vector.scalar_tensor_tensor(
                    fin["y"][:, mt, :], sg[:], -LAM3, featblk_sb[:, mt, :],
                    op0=ALU.mult, op1=ALU.add,
                )

            # ======================= FINAL: l2norm, logits, log_softmax =======================
            pT = st.tile([P, 3 * KF, B], bf16, name="pT", tag="pT")
            for xi, name in [(1, "z1"), (2, "z2"), (0, "y")]:
                x = fin[name]
                normed = wk.tile([P, MT, f], bf16, name="normed", tag="normed", bufs=1)
                for rc in range(MT):
                    sq = wk.tile([P, f], f32, name="sq", tag="lg")
                    ssq = wk.tile([P, 1], f32, name="ssq", tag="ssq")
                    nc.scalar.activation(sq[:], x[:, rc, :], AF.Square, accum_out=ssq[:])
                    nrm = wk.tile([P, 1], f32, name="nrm", tag="nrm")
                    nc.scalar.activation(nrm[:], ssq[:], AF.Sqrt)
                    nc.vector.tensor_scalar_max(nrm[:], nrm[:], 1e-12)
                    rn = wk.tile([P, 1], f32, name="rn", tag="rn")
                    nc.vector.reciprocal(rn[:], nrm[:])
                    nc.scalar.activation(normed[:, rc, :], x[:, rc, :], AF.Copy, scale=rn[:])
                # transpose normed [B, f] -> pT rows [f, B]
                for jf in range(KF):
                    psum = ps.tile([P, B], bf16, name="psum_tr", tag="psum")
                    for it in range(MT):
                        nc.tensor.transpose(
                            psum[:, it * P:(it + 1) * P],
                            normed[:, it, jf * P:(jf + 1) * P],
                            ident_sb[:],
                        )
                    nc.vector.tensor_copy(pT[:, xi * KF + jf, :], psum[:])

            w2T_re = dw2T.rearrange("(k p) c -> p k c", p=P)
            w2strips = []
            for j in range(3 * KF // SS):
                w2s = sp.tile([P, SS, f], bf16, name="w2s", tag="astrip")
                nc.sync.dma_start(w2s[:], w2T_re[:, j * SS:(j + 1) * SS, :])
                w2strips.append(w2s)
            for mt in range(MT):
                psum = ps.tile([P, f], f32, name="psum", tag="psum")
                for kc in range(3 * KF):
                    nc.tensor.matmul(psum[:], pT[:, kc, mt * P:(mt + 1) * P],
                                     w2strips[kc // SS][:, kc % SS, :],
                                     start=(kc == 0), stop=False)
                nc.tensor.matmul(psum[:], ones_sb[:], w2b_sb[:], start=False, stop=True)
                # log_softmax (fp32)
                L = wk.tile([P, f], f32, name="lg", tag="lg")
                nc.vector.tensor_copy(L[:], psum[:])
                nmx = wk.tile([P, 1], f32, name="nmx", tag="nmx")
                nc.vector.tensor_reduce(out=nmx[:], in_=L[:], op=ALU.max,
                                        axis=mybir.AxisListType.X, negate=True)
                ex = wk.tile([P, f], f32, name="ex", tag="ex")
                se = wk.tile([P, 1], f32, name="se", tag="se")
                nc.scalar.activation(ex[:], L[:], AF.Exp, bias=nmx[:], accum_out=se[:])
                lse = wk.tile([P, 1], f32, name="lse", tag="lse")
                nc.scalar.activation(lse[:], se[:], AF.Ln)
                b2 = wk.tile([P, 1], f32, name="b2", tag="b2")
                nc.vector.tensor_sub(b2[:], nmx[:], lse[:])
                lo = wk.tile([P, f], f32, name="lo", tag="lo")
                nc.scalar.activation(lo[:], L[:], AF.Identity, bias=b2[:])
                nc.sync.dma_start(dout[mt * P:(mt + 1) * P, :], lo[:])

    split_waits(nc, maxw=1)
    return nc


def prep_inputs(feat, adj, adj1, y, wy, w2_w, w2_b, n=N_FULL, ncores=NC):
    """Host-side sharding / transposition / casting. Returns in_maps."""
    B = n // ncores
    wy64 = np.asarray(wy, np.float64)
    WS = wy64 @ wy64.T + wy64.T @ wy64
    wy_bf = np.asarray(wy, BF)
    WS_bf = WS.astype(np.float32).astype(BF)
    w2T_bf = np.ascontiguousarray(np.asarray(w2_w, np.float32).T).astype(BF)
    w2b_bf = np.asarray(w2_b, np.float32).reshape(1, -1).astype(BF)
    ident = np.eye(P, dtype=np.float32).astype(BF)
    featfull_bf = np.asarray(feat, np.float32).astype(BF)
    in_maps = []
    for c in range(ncores):
        r0, r1 = c * B, (c + 1) * B
        in_maps.append({
            "yT": np.ascontiguousarray(np.asarray(y[r0:r1], np.float32).T).astype(BF),
            "featT": np.ascontiguousarray(np.asarray(feat[r0:r1], np.float32).T).astype(BF),
            "featblk": np.asarray(feat[r0:r1], np.float32).astype(BF),
            "featfull": featfull_bf,
            "adjT": np.ascontiguousarray(np.asarray(adj[r0:r1], np.float32).T).astype(BF),
            "adj1T": np.ascontiguousarray(np.asarray(adj1[r0:r1], np.float32).T).astype(BF),
            "wy": wy_bf,
            "WS": WS_bf,
            "w2T": w2T_bf,
            "w2b": w2b_bf,
            "ident": ident,
        })
    return in_maps


_CACHED_NC = None


def kernel(feat, adj, adj1, y, wy, w2_w, w2_b):
    global _CACHED_NC
    if _CACHED_NC is None:
        _CACHED_NC = build()
    in_maps = prep_inputs(feat, adj, adj1, y, wy, w2_w, w2_b)
    res = run_bass_kernel_spmd(_CACHED_NC, in_maps, core_ids=list(range(NC)))
    return np.concatenate([res.results[c]["out"] for c in range(NC)], axis=0)


# revision 18
# speedup vs baseline: 1.0692x; 1.0088x over previous
"""DGNN message-passing kernel for 8 TRN2 NeuronCores.

Row-shards the n dimension across 8 cores. Algebraic restructuring: the n x n
Gram matrices (temp11 = yw@yw.T - eps*z1w@z1w.T - ...) are never materialized.
Instead, for each sigmoid argument:

    (temp11 @ t)[rows_c, :] = sum_x coef_x * xw_x[rows_c, :] @ S_{x,t}
    with S_{x,t} = xw_x.T @ t = sum_c xw_x[rows_c].T @ t[rows_c]   (f x f)

The S matrices are computed as per-core block partials and AllReduced (f x f
each). The only O(n^2) matmuls left are adj @ z1 / adj1 @ z2, block-rows per
core. Between layers, z1/z2 row-blocks are AllGathered (y is only ever needed
block-wise). All heavy matmuls run in bf16 (fp32 PSUM accumulation); the
sigmoids saturate, final rel err vs fp64 reference ~1e-5.
"""

import numpy as np
import ml_dtypes

import concourse.bass as bass
import concourse.mybir as mybir
import concourse.tile as tile
from concourse.bass_utils import run_bass_kernel_spmd

BF = ml_dtypes.bfloat16
P = 128
N_FULL, F_FULL, NC = 4096, 512, 8
EPS = 0.5
LAM1 = 1.0
LAM2 = 1.0
LAM3 = 0.5
AF = mybir.ActivationFunctionType
ALU = mybir.AluOpType


def split_waits(nc, maxw=1):
    """This container's walrus build allows only `maxw` sync-wait commands per
    instruction. Split excess waits onto InstNoOps inserted just before, on the
    same engine (the engine executes the waits in order, so semantics are
    unchanged)."""
    for bb in nc.main_func.blocks:
        i = 0
        while i < len(bb.instructions):
            ins = bb.instructions[i]
            si = ins.sync_info
            if si and si.on_wait and len(si.on_wait) > maxw:
                waits = list(si.on_wait)
                extra, keep = waits[:-maxw], waits[-maxw:]
                si.on_wait = keep
                pos = i
                for j in range(0, len(extra), maxw):
                    nop = mybir.InstNoOp(name=f"wsplit_{ins.name}_{j}", ins=[], outs=[])
                    nop.engine = ins.engine
                    nop.sync_info = mybir.SyncInfo(on_wait=extra[j:j + maxw], on_update=[])
                    nc.register_instruction(nop)
                    bb.instructions.insert(pos, nop)
                    pos += 1
                    i += 1
            i += 1


def build(n=N_FULL, f=F_FULL, ncores=NC):
    B = n // ncores         # rows per core
    MT = B // P             # row tiles per block
    KF = f // P             # f chunks
    NKC = n // P            # node chunks
    SS = MT                 # node chunks per streamed super-strip (= rank block)
    NJ = NKC // SS
    f3 = 3 * f

    nc = bass.Bass("TRN2", target_bir_lowering=False, debug=False, num_devices=ncores)
    bf16, f32 = mybir.dt.bfloat16, mybir.dt.float32

    # ---- per-core DRAM I/O (host pre-shards / transposes / casts) ----
    dyT = nc.dram_tensor("yT", [f, B], bf16, kind="ExternalInput").ap()
    dfT = nc.dram_tensor("featT", [f, B], bf16, kind="ExternalInput").ap()
    dfeatblk = nc.dram_tensor("featblk", [B, f], bf16, kind="ExternalInput").ap()
    dfeatfull = nc.dram_tensor("featfull", [n, f], bf16, kind="ExternalInput").ap()
    dadjT = nc.dram_tensor("adjT", [n, B], bf16, kind="ExternalInput").ap()
    dadj1T = nc.dram_tensor("adj1T", [n, B], bf16, kind="ExternalInput").ap()
    dwy = nc.dram_tensor("wy", [f, f], bf16, kind="ExternalInput").ap()
    dWS = nc.dram_tensor("WS", [f, f], bf16, kind="ExternalInput").ap()
    dw2T = nc.dram_tensor("w2T", [f3, f], bf16, kind="ExternalInput").ap()
    dw2b = nc.dram_tensor("w2b", [1, f], bf16, kind="ExternalInput").ap()
    dident = nc.dram_tensor("ident", [P, P], bf16, kind="ExternalInput").ap()
    dout = nc.dram_tensor("out", [B, f], f32, kind="ExternalOutput").ap()

    with tile.TileContext(nc) as tc:
        with (
            tc.tile_pool(name="const", bufs=1) as cp,
            tc.tile_pool(name="fac", bufs=1) as fp,
            tc.tile_pool(name="stage", bufs=1) as st,
            tc.tile_pool(name="work", bufs=2) as wk,
            tc.tile_pool(name="strips", bufs=4) as sp,
            tc.tile_pool(name="ps", bufs=8, space="PSUM") as ps,
            tc.tile_pool(name="dram", bufs=1, space="DRAM") as dram,
        ):
            # ---------------- constant loads ----------------
            def load_chunked(dst_name, src, nch, width, dt=bf16, pool=cp):
                t = pool.tile([P, nch, width], dt, name=dst_name, tag=dst_name)
                nc.sync.dma_start(t[:], src.rearrange("(k p) c -> p k c", p=P))
                return t

            wy_sb = load_chunked("wy_sb", dwy, KF, f)
            WS_sb = load_chunked("WS_sb", dWS, KF, f)
            yT0 = load_chunked("yT0", dyT, KF, B)
            fT0 = load_chunked("fT0", dfT, KF, B)
            featblk_sb = load_chunked("featblk_sb", dfeatblk, MT, f)
            ident_sb = cp.tile([P, P], bf16, name="ident_sb", tag="ident_sb")
            nc.sync.dma_start(ident_sb[:], dident[:])
            w2b_sb = cp.tile([1, f], bf16, name="w2b_sb", tag="w2b_sb")
            nc.sync.dma_start(w2b_sb[:], dw2b[:])
            ones_sb = cp.tile([1, P], bf16, name="ones_sb", tag="ones_sb")
            nc.vector.memset(ones_sb[:], 1.0)

            # AllGather buffers for the layer boundary (z1, z2 blocks)
            ag_in = dram.tile([2 * B, f], bf16, name="ag_in")
            ag_out = dram.tile([2 * n, f], bf16, name="ag_out", addr_space="Shared")
            # ag_out rows: [core c][z][kc within block][p]
            ag_view = ag_out[:].rearrange(
                "(c z kc p) col -> z p c kc col", c=ncores, z=2, p=P
            )

            # ---------------- helpers ----------------
            def mm_block(dst, lhsT, rhs_sb, out_dt=bf16, eng="v"):
                """dst[128, MT_or_KF, width] = lhsT.T @ rhs ; lhsT/rhs chunked [128,KF,*]."""
                nmt = dst.shape[1]
                for mt in range(nmt):
                    psum = ps.tile([P, rhs_sb.shape[2]], f32, name="psum", tag="psum")
                    for kc in range(KF):
                        nc.tensor.matmul(
                            psum[:],
                            lhsT[:, kc, mt * P:(mt + 1) * P],
                            rhs_sb[:, kc, :],
                            start=(kc == 0),
                            stop=(kc == KF - 1),
                        )
                    if eng == "v":
                        nc.vector.tensor_copy(dst[:, mt, :], psum[:])
                    else:
                        nc.scalar.activation(dst[:, mt, :], psum[:], AF.Copy)

            def layer(xTs, temps_of, S_groups, zsrc_views, outs_cfg, lnum):
                """One DGNN layer.
                xTs: dict name -> xT tile ([128,KF,B], lhsT blocks of y/z1/z2)
                temps_of: list of (temp_name, x_name) for t = x @ WS
                S_groups: list of groups of (x_name, temp_name, coef); one
                    AllReduce per group, kicked right after its partials
                zsrc_views: list of (adjT dram, z dram view or None->featfull) for the 2 adj mms
                outs_cfg: dict describing output handling (differs l0/l1)
                """
                names = list(xTs.keys())
                # factors, normal layout: xw = x @ wy ; t = x @ WS
                # (xwT is NOT needed for the S partials -> computed after the
                # AllReduces are kicked, so the collectives start ASAP)
                xw = {}
                tt = {}
                xwT = {}
                for x in names:
                    xw[x] = fp.tile([P, MT, f], bf16, name=f"xw_{x}", tag=f"xw_{x}")
                    mm_block(xw[x], xTs[x], wy_sb)

                # S partials with coefficient folded in (bf16 AllReduce), split
                # into groups: the z-path group (t24/t34) is reduced first so
                # its sigmoid matmuls unblock earlier.
                temps_x = dict((t, x) for t, x in temps_of)
                S_lookup = {}
                for gi, group in enumerate(S_groups):
                    for tname in sorted({t for (_, t, _) in group}):
                        if tname not in tt:
                            tt[tname] = fp.tile([P, MT, f], bf16,
                                                name=f"tt_{tname}", tag=f"tt_{tname}")
                            mm_block(tt[tname], xTs[temps_x[tname]], WS_sb)
                    nS = len(group)
                    S_stage = st.tile([P, nS * KF, f], bf16,
                                      name=f"S_stage_{gi}", tag=f"S_stage_{gi}")
                    for si, (x, tname, coef) in enumerate(group):
                        for mt in range(KF):  # m dim of S is f
                            psum = ps.tile([P, f], f32, name="psum", tag="psum")
                            for kc in range(MT):  # contraction over rows of block
                                nc.tensor.matmul(
                                    psum[:],
                                    xw[x][:, kc, mt * P:(mt + 1) * P],
                                    tt[tname][:, kc, :],
                                    start=(kc == 0),
                                    stop=(kc == MT - 1),
                                )
                            nc.scalar.activation(
                                S_stage[:, si * KF + mt, :], psum[:], AF.Copy,
                                scale=float(coef)
                            )
                    ar_in = dram.tile([nS * f, f], bf16, name=f"ar_in_{lnum}_{gi}")
                    ar_out = dram.tile([nS * f, f], bf16, name=f"ar_out_{lnum}_{gi}",
                                       addr_space="Shared")
                    nc.gpsimd.dma_start(ar_in[:].rearrange("(k p) c -> p k c", p=P), S_stage[:])
                    nc.gpsimd.collective_compute(
                        "AllReduce", ALU.add,
                        replica_groups=[list(range(ncores))],
                        ins=[ar_in[:].opt()], outs=[ar_out[:].opt()],
                    )
                    S_sb = st.tile([P, nS * KF, f], bf16,
                                   name=f"S_sb_{gi}", tag=f"S_stage_{gi}")
                    nc.gpsimd.dma_start(S_sb[:], ar_out[:].rearrange("(k p) c -> p k c", p=P))
                    for si, (x, t, _) in enumerate(group):
                        S_lookup[(x, t)] = (S_sb, si)

                # xwT = xw.T via PE transposes (cheaper than matmul, runs during AR)
                for x in names:
                    xwT[x] = fp.tile([P, KF, B], bf16, name=f"xwT_{x}", tag=f"xwT_{x}")
                    for jf in range(KF):
                        pst = ps.tile([P, B], bf16, name="psum_tr", tag="psum")
                        for it in range(MT):
                            nc.tensor.transpose(
                                pst[:, it * P:(it + 1) * P],
                                xw[x][:, it, jf * P:(jf + 1) * P],
                                ident_sb[:],
                            )
                        nc.vector.tensor_copy(xwT[x][:, jf, :], pst[:])

                def sig_mm(tname, transposed, dst, dst_sl, out_dt):
                    """sigmoid(sum_x xw_x @ S_{x,t}) into dst[:, dst_sl, :].
                    transposed=False: out[r, fo'] via lhsT=xwT ; True: out[fo', r] via lhsT=S."""
                    pairs = [(x, t) for g in S_groups for (x, t, _) in g if t == tname]
                    nmt = MT if not transposed else KF
                    res = []
                    for mt in range(nmt):
                        psum = ps.tile([P, B if transposed else f], f32, name="psum", tag="psum")
                        k = 0
                        nmm = len(pairs) * KF
                        for (x, t) in pairs:
                            sb, si = S_lookup[(x, t)]
                            for kc in range(KF):
                                if not transposed:
                                    lhsT = xwT[x][:, kc, mt * P:(mt + 1) * P]
                                    rhs = sb[:, si * KF + kc, :]
                                else:
                                    lhsT = sb[:, si * KF + kc, mt * P:(mt + 1) * P]
                                    rhs = xwT[x][:, kc, :]
                                nc.tensor.matmul(psum[:], lhsT, rhs,
                                                 start=(k == 0), stop=(k == nmm - 1))
                                k += 1
                        res.append(psum)
                    return res  # list of PSUM tiles (pre-sigmoid)

                # adj matmuls, normal layout: out[r, f] ; lhsT = adjT strips, rhs = z strips
                def adj_mm(dadj, z_strip_of):
                    psums = [ps.tile([P, f], f32, name="psum_adj", tag="psum")
                             for _ in range(MT)]
                    adj_re = dadj.rearrange("(k p) r -> p k r", p=P)
                    for j in range(NJ):
                        astrip = sp.tile([P, SS, B], bf16, name="astrip", tag="astrip")
                        nc.sync.dma_start(astrip[:], adj_re[:, j * SS:(j + 1) * SS, :])
                        zstrip = sp.tile([P, SS, f], bf16, name="zstrip", tag="zstrip")
                        nc.sync.dma_start(zstrip[:], z_strip_of(j))
                        for q in range(SS):
                            kc = j * SS + q
                            for mt in range(MT):
                                nc.tensor.matmul(
                                    psums[mt][:],
                                    astrip[:, q, mt * P:(mt + 1) * P],
                                    zstrip[:, q, :],
                                    start=(kc == 0),
                                    stop=(kc == NKC - 1),
                                )
                    return psums

                return xw, xwT, tt, sig_mm, adj_mm

            # ======================= LAYER 0 (z1 = z2 = feat) =======================
            xTs0 = {"y": yT0, "z1": fT0}
            temps0 = [("t12", "y"), ("t24", "z1")]
            S0 = [[("y", "t24", -1.0), ("z1", "t24", 1.0),
                   ("y", "t12", 1.0), ("z1", "t12", -1.0)]]
            featfull_view = dfeatfull.rearrange("(k p) c -> p k c", p=P)
            xw0, xwT0, tt0, sig_mm0, adj_mm0 = layer(xTs0, temps0, S0, None, None, 0)

            # adj matmuls first: they do not depend on the AllReduce, so the PE
            # works on them while the collective is in flight. Copy PSUM->SBUF
            # to free banks for the sigmoid matmuls.
            adj_out0 = []
            for ai, dadj in enumerate((dadjT, dadj1T)):
                psums = adj_mm0(dadj, lambda j: featfull_view[:, j * SS:(j + 1) * SS, :])
                ao = st.tile([P, MT, f], bf16, name=f"adj_out{ai}", tag=f"adj_out{ai}")
                for mt in range(MT):
                    nc.vector.tensor_copy(ao[:, mt, :], psums[mt][:])
                adj_out0.append(ao)

            # sig2 (normal layout), shared by z1_n and z2_n at layer 0
            pre2 = sig_mm0("t24", False, None, None, None)
            sig2_0 = st.tile([P, MT, f], bf16, name="sig2_0", tag="sig2_0")
            for mt in range(MT):
                nc.scalar.activation(sig2_0[:, mt, :], pre2[mt][:], AF.Sigmoid)

            # z1_n = adj@feat - 0.25*sig2 ; z2_n = adj1@feat - 0.25*sig2  (normal layout)
            z1s = st.tile([P, MT, f], bf16, name="z1s", tag="z1s")
            z2s = st.tile([P, MT, f], bf16, name="z2s", tag="z2s")
            for (ao, dst, cf) in [(adj_out0[0], z1s, EPS * LAM3 / LAM1),
                                  (adj_out0[1], z2s, (1.0 - EPS) * LAM3 / LAM2)]:
                for mt in range(MT):
                    nc.vector.scalar_tensor_tensor(
                        dst[:, mt, :], sig2_0[:, mt, :], -cf, ao[:, mt, :],
                        op0=ALU.mult, op1=ALU.add,
                    )

            # AllGather z1/z2 blocks -> full (normal layout) for layer-1 adj matmuls
            nc.gpsimd.dma_start(ag_in[0:B, :].rearrange("(k p) c -> p k c", p=P), z1s[:])
            nc.gpsimd.dma_start(ag_in[B:2 * B, :].rearrange("(k p) c -> p k c", p=P), z2s[:])
            nc.gpsimd.collective_compute(
                "AllGather", ALU.bypass,
                replica_groups=[list(range(ncores))],
                ins=[ag_in[:].opt()], outs=[ag_out[:].opt()],
            )

            # y_nT = featT - 0.5*sigmoid(pre1T)   (T layout -> next-layer lhsT directly)
            yT2 = st.tile([P, KF, B], bf16, name="yT2", tag="yT2")
            pre1T = sig_mm0("t12", True, None, None, None)
            for mt in range(KF):
                sg = wk.tile([P, B], bf16, name="sg_t", tag="sg_t")
                nc.scalar.activation(sg[:], pre1T[mt][:], AF.Sigmoid)
                nc.vector.scalar_tensor_tensor(
                    yT2[:, mt, :], sg[:], -LAM3, fT0[:, mt, :], op0=ALU.mult, op1=ALU.add
                )

            # local transposes: z1T2/z2T2 = (z1_n block).T for layer-1 factor lhsT
            z1T2 = st.tile([P, KF, B], bf16, name="z1T2", tag="z1T2")
            z2T2 = st.tile([P, KF, B], bf16, name="z2T2", tag="z2T2")
            for (src, dst) in [(z1s, z1T2), (z2s, z2T2)]:
                for jf in range(KF):
                    psum = ps.tile([P, B], bf16, name="psum_tr", tag="psum")
                    for it in range(MT):
                        nc.tensor.transpose(
                            psum[:, it * P:(it + 1) * P],
                            src[:, it, jf * P:(jf + 1) * P],
                            ident_sb[:],
                        )
                    nc.vector.tensor_copy(dst[:, jf, :], psum[:])

            # ======================= LAYER 1 =======================
            xTs1 = {"y": yT2, "z1": z1T2, "z2": z2T2}
            temps1 = [("t12", "y"), ("t24", "z1"), ("t34", "z2")]
            S1 = [[("y", "t24", -1.0), ("z1", "t24", EPS), ("z2", "t24", (1 - EPS)),
                   ("y", "t34", -1.0), ("z1", "t34", EPS), ("z2", "t34", (1 - EPS))],
                  [("y", "t12", 1.0), ("z1", "t12", -EPS), ("z2", "t12", -(1 - EPS))]]
            xw1, xwT1, tt1, sig_mm1, adj_mm1 = layer(xTs1, temps1, S1, None, None, 1)

            # final y/z1/z2 blocks in fp32 (normal layout)
            fin = {}
            for name in ("y", "z1", "z2"):
                fin[name] = st.tile([P, MT, f], bf16, name=f"fin_{name}", tag=f"fin_{name}")

            # adj matmuls first (depend on the AllGather, not the AllReduce)
            adj_out1 = []
            for ai, (dadj, zi) in enumerate(((dadjT, 0), (dadj1T, 1))):
                psums = adj_mm1(dadj, lambda j, z=zi: ag_view[z][:, j, :, :])
                ao = st.tile([P, MT, f], bf16, name=f"adj_out{ai}", tag=f"adj_out{ai}")
                for mt in range(MT):
                    nc.vector.tensor_copy(ao[:, mt, :], psums[mt][:])
                adj_out1.append(ao)

            for (tname, ao, dst, cf) in [
                ("t24", adj_out1[0], fin["z1"], EPS * LAM3 / LAM1),
                ("t34", adj_out1[1], fin["z2"], (1 - EPS) * LAM3 / LAM2),
            ]:
                pre = sig_mm1(tname, False, None, None, None)
                for mt in range(MT):
                    sg = wk.tile([P, f], f32, name="sg_f", tag="sg_f")
                    nc.scalar.activation(sg[:], pre[mt][:], AF.Sigmoid)
                    nc.vector.scalar_tensor_tensor(
                        dst[:, mt, :], sg[:], -cf, ao[:, mt, :],
                        op0=ALU.mult, op1=ALU.add,
                    )

            pre1 = sig_mm1("t12", False, None, None, None)
            for mt in range(MT):
                sg = wk.tile([P, f], f32, name="sg_f", tag="sg_f")
                nc.scalar.activation(sg[:], pre1[mt][:], AF.Sigmoid)
                nc.vector.scalar_tensor_tensor(
                    fin["y"][:, mt, :], sg[:], -LAM3, featblk_sb[:, mt, :],
                    op0=ALU.mult, op1=ALU.add,
                )

            # ======================= FINAL: l2norm, logits, log_softmax =======================
            pT = st.tile([P, 3 * KF, B], bf16, name="pT", tag="pT")
            for xi, name in [(1, "z1"), (2, "z2"), (0, "y")]:
                x = fin[name]
                normed = wk.tile([P, MT, f], bf16, name="normed", tag="normed", bufs=1)
                for rc in range(MT):
                    sq = wk.tile([P, f], f32, name="sq", tag="lg")
                    ssq = wk.tile([P, 1], f32, name="ssq", tag="ssq")
                    nc.scalar.activation(sq[:], x[:, rc, :], AF.Square, accum_out=ssq[:])
                    nrm = wk.tile([P, 1], f32, name="nrm", tag="nrm")
                    nc.scalar.activation(nrm[:], ssq[:], AF.Sqrt)
                    nc.vector.tensor_scalar_max(nrm[:], nrm[:], 1e-12)
                    rn = wk.tile([P, 1], f32, name="rn", tag="rn")
                    nc.vector.reciprocal(rn[:], nrm[:])
                    nc.scalar.activation(normed[:, rc, :], x[:, rc, :], AF.Copy, scale=rn[:])
                # transpose normed [B, f] -> pT rows [f, B]
                for jf in range(KF):
                    psum = ps.tile([P, B], bf16, name="psum_tr", tag="psum")
                    for it in range(MT):
                        nc.tensor.transpose(
                            psum[:, it * P:(it + 1) * P],
                            normed[:, it, jf * P:(jf + 1) * P],
                            ident_sb[:],
                        )
                    nc.vector.tensor_copy(pT[:, xi * KF + jf, :], psum[:])

            w2T_re = dw2T.rearrange("(k p) c -> p k c", p=P)
            w2strips = []
            for j in range(3 * KF // SS):
                w2s = sp.tile([P, SS, f], bf16, name="w2s", tag="astrip")
                nc.sync.dma_start(w2s[:], w2T_re[:, j * SS:(j + 1) * SS, :])
                w2strips.append(w2s)
            for mt in range(MT):
                psum = ps.tile([P, f], f32, name="psum", tag="psum")
                for kc in range(3 * KF):
                    nc.tensor.matmul(psum[:], pT[:, kc, mt * P:(mt + 1) * P],
                                     w2strips[kc // SS][:, kc % SS, :],
                                     start=(kc == 0), stop=False)
                nc.tensor.matmul(psum[:], ones_sb[:], w2b_sb[:], start=False, stop=True)
                # log_softmax (fp32)
                L = wk.tile([P, f], f32, name="lg", tag="lg")
                nc.vector.tensor_copy(L[:], psum[:])
                nmx = wk.tile([P, 1], f32, name="nmx", tag="nmx")
                nc.vector.tensor_reduce(out=nmx[:], in_=L[:], op=ALU.max,
                                        axis=mybir.AxisListType.X, negate=True)
                ex = wk.tile([P, f], f32, name="ex", tag="ex")
                se = wk.tile([P, 1], f32, name="se", tag="se")
                nc.scalar.activation(ex[:], L[:], AF.Exp, bias=nmx[:], accum_out=se[:])
                lse = wk.tile([P, 1], f32, name="lse", tag="lse")
                nc.scalar.activation(lse[:], se[:], AF.Ln)
                b2 = wk.tile([P, 1], f32, name="b2", tag="b2")
                nc.vector.tensor_sub(b2[:], nmx[:], lse[:])
                lo = wk.tile([P, f], f32, name="lo", tag="lo")
                nc.scalar.activation(lo[:], L[:], AF.Identity, bias=b2[:])
                nc.sync.dma_start(dout[mt * P:(mt + 1) * P, :], lo[:])

    split_waits(nc, maxw=1)
    return nc


def prep_inputs(feat, adj, adj1, y, wy, w2_w, w2_b, n=N_FULL, ncores=NC):
    """Host-side sharding / transposition / casting. Returns in_maps."""
    B = n // ncores
    wy64 = np.asarray(wy, np.float64)
    WS = wy64 @ wy64.T + wy64.T @ wy64
    wy_bf = np.asarray(wy, BF)
    WS_bf = WS.astype(np.float32).astype(BF)
    w2T_bf = np.ascontiguousarray(np.asarray(w2_w, np.float32).T).astype(BF)
    w2b_bf = np.asarray(w2_b, np.float32).reshape(1, -1).astype(BF)
    ident = np.eye(P, dtype=np.float32).astype(BF)
    featfull_bf = np.asarray(feat, np.float32).astype(BF)
    in_maps = []
    for c in range(ncores):
        r0, r1 = c * B, (c + 1) * B
        in_maps.append({
            "yT": np.ascontiguousarray(np.asarray(y[r0:r1], np.float32).T).astype(BF),
            "featT": np.ascontiguousarray(np.asarray(feat[r0:r1], np.float32).T).astype(BF),
            "featblk": np.asarray(feat[r0:r1], np.float32).astype(BF),
            "featfull": featfull_bf,
            "adjT": np.ascontiguousarray(np.asarray(adj[r0:r1], np.float32).T).astype(BF),
            "adj1T": np.ascontiguousarray(np.asarray(adj1[r0:r1], np.float32).T).astype(BF),
            "wy": wy_bf,
            "WS": WS_bf,
            "w2T": w2T_bf,
            "w2b": w2b_bf,
            "ident": ident,
        })
    return in_maps


_CACHED_NC = None


def kernel(feat, adj, adj1, y, wy, w2_w, w2_b):
    global _CACHED_NC
    if _CACHED_NC is None:
        _CACHED_NC = build()
    in_maps = prep_inputs(feat, adj, adj1, y, wy, w2_w, w2_b)
    res = run_bass_kernel_spmd(_CACHED_NC, in_maps, core_ids=list(range(NC)))
    return np.concatenate([res.results[c]["out"] for c in range(NC)], axis=0)


# revision 19
# speedup vs baseline: 1.1445x; 1.0704x over previous
"""DGNN message-passing kernel for 8 TRN2 NeuronCores.

Row-shards the n dimension across 8 cores. Algebraic restructuring: the n x n
Gram matrices (temp11 = yw@yw.T - eps*z1w@z1w.T - ...) are never materialized.
Instead, for each sigmoid argument:

    (temp11 @ t)[rows_c, :] = sum_x coef_x * xw_x[rows_c, :] @ S_{x,t}
    with S_{x,t} = xw_x.T @ t = sum_c xw_x[rows_c].T @ t[rows_c]   (f x f)

The S matrices are computed as per-core block partials and AllReduced (f x f
each, coefficient pre-folded, bf16). The only O(n^2) matmuls left are
adj @ z1 / adj1 @ z2, block-rows per core; between layers z1/z2 row-blocks are
AllGathered (y is only ever needed block-wise). All heavy matmuls run in bf16
with fp32 PSUM accumulation; final rel err vs the fp32 reference ~1e-5.

All DRAM tensors (inputs and collective bounce buffers) use an "SBUF image"
layout [128, k*width]: row p holds every chunk's partition-p row back to back,
so every DMA is per-partition contiguous (1 descriptor/partition instead of
one per 1-2KB chunk). The host pre-shards, transposes, casts and images the
inputs; collectives are elementwise/concat so the image layout is transparent
to them.
"""

import numpy as np
import ml_dtypes

import concourse.bass as bass
import concourse.mybir as mybir
import concourse.tile as tile
from concourse.bass_utils import run_bass_kernel_spmd

BF = ml_dtypes.bfloat16
P = 128
N_FULL, F_FULL, NC = 4096, 512, 8
EPS = 0.5
LAM1 = 1.0
LAM2 = 1.0
LAM3 = 0.5
AF = mybir.ActivationFunctionType
ALU = mybir.AluOpType


def split_waits(nc, maxw=1):
    """This container's walrus build allows only `maxw` sync-wait commands per
    instruction. Split excess waits onto InstNoOps inserted just before, on the
    same engine (the engine executes the waits in order, so semantics are
    unchanged)."""
    for bb in nc.main_func.blocks:
        i = 0
        while i < len(bb.instructions):
            ins = bb.instructions[i]
            si = ins.sync_info
            if si and si.on_wait and len(si.on_wait) > maxw:
                waits = list(si.on_wait)
                extra, keep = waits[:-maxw], waits[-maxw:]
                si.on_wait = keep
                pos = i
                for j in range(0, len(extra), maxw):
                    nop = mybir.InstNoOp(name=f"wsplit_{ins.name}_{j}", ins=[], outs=[])
                    nop.engine = ins.engine
                    nop.sync_info = mybir.SyncInfo(on_wait=extra[j:j + maxw], on_update=[])
                    nc.register_instruction(nop)
                    bb.instructions.insert(pos, nop)
                    pos += 1
                    i += 1
            i += 1


def build(n=N_FULL, f=F_FULL, ncores=NC):
    B = n // ncores         # rows per core
    MT = B // P             # row tiles per block
    KF = f // P             # f chunks
    NKC = n // P            # node chunks
    SS = MT                 # node chunks per streamed super-strip (= rank block)
    NJ = NKC // SS

    nc = bass.Bass("TRN2", target_bir_lowering=False, debug=False, num_devices=ncores)
    bf16, f32 = mybir.dt.bfloat16, mybir.dt.float32

    # ---- per-core DRAM I/O, all in SBUF-image layout [128, k*width] ----
    def dimg(name, nch, width, dt=bf16):
        t = nc.dram_tensor(name, [P, nch * width], dt, kind="ExternalInput").ap()
        return t.rearrange("p (k c) -> p k c", k=nch)

    dyT = dimg("yT", KF, B)
    dfT = dimg("featT", KF, B)
    dfeatblk = dimg("featblk", MT, f)
    dfeatfull = dimg("featfull", NKC, f)
    dadjT = dimg("adjT", NKC, B)
    dadj1T = dimg("adj1T", NKC, B)
    dwy = dimg("wy", KF, f)
    dWS = dimg("WS", KF, f)
    dw2T = dimg("w2T", 3 * KF, f)
    dw2b = nc.dram_tensor("w2b", [1, f], bf16, kind="ExternalInput").ap()
    dident = nc.dram_tensor("ident", [P, P], bf16, kind="ExternalInput").ap()
    dout = nc.dram_tensor("out", [B, f], f32, kind="ExternalOutput").ap()

    with tile.TileContext(nc) as tc:
        with (
            tc.tile_pool(name="const", bufs=1) as cp,
            tc.tile_pool(name="fac", bufs=1) as fp,
            tc.tile_pool(name="stage", bufs=1) as st,
            tc.tile_pool(name="work", bufs=2) as wk,
            tc.tile_pool(name="strips", bufs=4) as sp,
            tc.tile_pool(name="ps", bufs=8, space="PSUM") as ps,
            tc.tile_pool(name="dram", bufs=1, space="DRAM") as dram,
        ):
            # ---------------- constant loads ----------------
            def load_img(dst_name, src, nch, width, dt=bf16, pool=cp):
                t = pool.tile([P, nch, width], dt, name=dst_name, tag=dst_name)
                nc.sync.dma_start(t[:], src)
                return t

            wy_sb = load_img("wy_sb", dwy, KF, f)
            WS_sb = load_img("WS_sb", dWS, KF, f)
            yT0 = load_img("yT0", dyT, KF, B)
            fT0 = load_img("fT0", dfT, KF, B)
            featblk_sb = load_img("featblk_sb", dfeatblk, MT, f)
            ident_sb = cp.tile([P, P], bf16, name="ident_sb", tag="ident_sb")
            nc.sync.dma_start(ident_sb[:], dident[:])
            w2b_sb = cp.tile([1, f], bf16, name="w2b_sb", tag="w2b_sb")
            nc.sync.dma_start(w2b_sb[:], dw2b[:])
            ones_sb = cp.tile([1, P], bf16, name="ones_sb", tag="ones_sb")
            nc.vector.memset(ones_sb[:], 1.0)

            # AllGather buffers (SBUF-image layout; AllGather = concat of rank
            # images along rows, so rank j's block is rows [j*128,(j+1)*128))
            ag_in = dram.tile([P, 2 * MT * f], bf16, name="ag_in")
            ag_out = dram.tile([ncores * P, 2 * MT * f], bf16, name="ag_out",
                               addr_space="Shared")

            def ag_strip(zi, j):
                return ag_out[j * P:(j + 1) * P,
                              zi * MT * f:(zi + 1) * MT * f].rearrange(
                                  "p (k c) -> p k c", k=MT)

            # ---------------- helpers ----------------
            def mm_block(dst, lhsT, rhs_sb):
                """dst[128, nmt, width] = lhsT.T @ rhs ; lhsT/rhs chunked [128,KF,*]."""
                nmt = dst.shape[1]
                for mt in range(nmt):
                    psum = ps.tile([P, rhs_sb.shape[2]], f32, name="psum", tag="psum")
                    for kc in range(KF):
                        nc.tensor.matmul(
                            psum[:],
                            lhsT[:, kc, mt * P:(mt + 1) * P],
                            rhs_sb[:, kc, :],
                            start=(kc == 0),
                            stop=(kc == KF - 1),
                        )
                    nc.vector.tensor_copy(dst[:, mt, :], psum[:])

            def layer(xTs, temps_of, S_groups, lnum):
                """One DGNN layer: factors, S partials (+ one AllReduce per
                group, kicked right after its partials), xwT transposes."""
                names = list(xTs.keys())
                xw = {}
                tt = {}
                xwT = {}
                for x in names:
                    xw[x] = fp.tile([P, MT, f], bf16, name=f"xw_{x}", tag=f"xw_{x}")
                    mm_block(xw[x], xTs[x], wy_sb)

                temps_x = dict((t, x) for t, x in temps_of)
                S_lookup = {}
                for gi, group in enumerate(S_groups):
                    for tname in sorted({t for (_, t, _) in group}):
                        if tname not in tt:
                            tt[tname] = fp.tile([P, MT, f], bf16,
                                                name=f"tt_{tname}", tag=f"tt_{tname}")
                            mm_block(tt[tname], xTs[temps_x[tname]], WS_sb)
                    nS = len(group)
                    S_stage = st.tile([P, nS * KF, f], bf16,
                                      name=f"S_stage_{gi}", tag=f"S_stage_{gi}")
                    for si, (x, tname, coef) in enumerate(group):
                        for mt in range(KF):  # m dim of S is f
                            psum = ps.tile([P, f], f32, name="psum", tag="psum")
                            for kc in range(MT):  # contraction over block rows
                                nc.tensor.matmul(
                                    psum[:],
                                    xw[x][:, kc, mt * P:(mt + 1) * P],
                                    tt[tname][:, kc, :],
                                    start=(kc == 0),
                                    stop=(kc == MT - 1),
                                )
                            nc.scalar.activation(
                                S_stage[:, si * KF + mt, :], psum[:], AF.Copy,
                                scale=float(coef)
                            )
                    ar_in = dram.tile([P, nS * KF * f], bf16, name=f"ar_in_{lnum}_{gi}")
                    ar_out = dram.tile([P, nS * KF * f], bf16, name=f"ar_out_{lnum}_{gi}",
                                       addr_space="Shared")
                    ar_in_v = ar_in[:].rearrange("p (k c) -> p k c", k=nS * KF)
                    ar_out_v = ar_out[:].rearrange("p (k c) -> p k c", k=nS * KF)
                    nc.gpsimd.dma_start(ar_in_v, S_stage[:])
                    nc.gpsimd.collective_compute(
                        "AllReduce", ALU.add,
                        replica_groups=[list(range(ncores))],
                        ins=[ar_in[:].opt()], outs=[ar_out[:].opt()],
                    )
                    S_sb = st.tile([P, nS * KF, f], bf16,
                                   name=f"S_sb_{gi}", tag=f"S_stage_{gi}")
                    nc.gpsimd.dma_start(S_sb[:], ar_out_v)
                    for si, (x, t, _) in enumerate(group):
                        S_lookup[(x, t)] = (S_sb, si)

                # xwT = xw.T via PE transposes (cheap, runs during the AR)
                for x in names:
                    xwT[x] = fp.tile([P, KF, B], bf16, name=f"xwT_{x}", tag=f"xwT_{x}")
                    for jf in range(KF):
                        pst = ps.tile([P, B], bf16, name="psum_tr", tag="psum")
                        for it in range(MT):
                            nc.tensor.transpose(
                                pst[:, it * P:(it + 1) * P],
                                xw[x][:, it, jf * P:(jf + 1) * P],
                                ident_sb[:],
                            )
                        nc.vector.tensor_copy(xwT[x][:, jf, :], pst[:])

                def sig_mm(tname, transposed):
                    """PSUM tiles of sum_x coef*xw_x @ S_{x,t} (pre-sigmoid).
                    transposed=False: out[r, fo'] via lhsT=xwT ; True: out[fo', r]
                    via lhsT=S."""
                    pairs = [(x, t) for g in S_groups for (x, t, _) in g if t == tname]
                    nmt = MT if not transposed else KF
                    res = []
                    for mt in range(nmt):
                        psum = ps.tile([P, B if transposed else f], f32,
                                       name="psum", tag="psum")
                        k = 0
                        nmm = len(pairs) * KF
                        for (x, t) in pairs:
                            sb, si = S_lookup[(x, t)]
                            for kc in range(KF):
                                if not transposed:
                                    lhsT = xwT[x][:, kc, mt * P:(mt + 1) * P]
                                    rhs = sb[:, si * KF + kc, :]
                                else:
                                    lhsT = sb[:, si * KF + kc, mt * P:(mt + 1) * P]
                                    rhs = xwT[x][:, kc, :]
                                nc.tensor.matmul(psum[:], lhsT, rhs,
                                                 start=(k == 0), stop=(k == nmm - 1))
                                k += 1
                        res.append(psum)
                    return res

                def adj_mm(dadj, z_strip_of):
                    """adj @ z row-block, normal layout [r, f]."""
                    psums = [ps.tile([P, f], f32, name="psum_adj", tag="psum")
                             for _ in range(MT)]
                    for j in range(NJ):
                        astrip = sp.tile([P, SS, B], bf16, name="astrip", tag="astrip")
                        nc.sync.dma_start(astrip[:], dadj[:, j * SS:(j + 1) * SS, :])
                        zstrip = sp.tile([P, SS, f], bf16, name="zstrip", tag="zstrip")
                        nc.sync.dma_start(zstrip[:], z_strip_of(j))
                        for q in range(SS):
                            kc = j * SS + q
                            for mt in range(MT):
                                nc.tensor.matmul(
                                    psums[mt][:],
                                    astrip[:, q, mt * P:(mt + 1) * P],
                                    zstrip[:, q, :],
                                    start=(kc == 0),
                                    stop=(kc == NKC - 1),
                                )
                    return psums

                return xw, xwT, sig_mm, adj_mm

            # ======================= LAYER 0 (z1 = z2 = feat) =======================
            xTs0 = {"y": yT0, "z1": fT0}
            temps0 = [("t12", "y"), ("t24", "z1")]
            S0 = [[("y", "t24", -1.0), ("z1", "t24", 1.0),
                   ("y", "t12", 1.0), ("z1", "t12", -1.0)]]
            xw0, xwT0, sig_mm0, adj_mm0 = layer(xTs0, temps0, S0, 0)

            # adj matmuls overlap the AllReduce; copy PSUM->SBUF to free banks
            adj_out0 = []
            for ai, dadj in enumerate((dadjT, dadj1T)):
                psums = adj_mm0(dadj, lambda j: dfeatfull[:, j * SS:(j + 1) * SS, :])
                ao = st.tile([P, MT, f], bf16, name=f"adj_out{ai}", tag=f"adj_out{ai}")
                for mt in range(MT):
                    nc.vector.tensor_copy(ao[:, mt, :], psums[mt][:])
                adj_out0.append(ao)

            # sig2 (normal layout), shared by z1_n and z2_n at layer 0
            pre2 = sig_mm0("t24", False)
            sig2_0 = st.tile([P, MT, f], bf16, name="sig2_0", tag="sig2_0")
            for mt in range(MT):
                nc.scalar.activation(sig2_0[:, mt, :], pre2[mt][:], AF.Sigmoid)

            # z1_n = adj@feat - 0.25*sig2 ; z2_n = adj1@feat - 0.25*sig2
            z1s = st.tile([P, MT, f], bf16, name="z1s", tag="z1s")
            z2s = st.tile([P, MT, f], bf16, name="z2s", tag="z2s")
            for (ao, dst, cf) in [(adj_out0[0], z1s, EPS * LAM3 / LAM1),
                                  (adj_out0[1], z2s, (1.0 - EPS) * LAM3 / LAM2)]:
                for mt in range(MT):
                    nc.vector.scalar_tensor_tensor(
                        dst[:, mt, :], sig2_0[:, mt, :], -cf, ao[:, mt, :],
                        op0=ALU.mult, op1=ALU.add,
                    )

            # AllGather z1/z2 blocks -> full (for layer-1 adj matmuls)
            nc.gpsimd.dma_start(
                ag_in[:, 0:MT * f].rearrange("p (k c) -> p k c", k=MT), z1s[:])
            nc.gpsimd.dma_start(
                ag_in[:, MT * f:2 * MT * f].rearrange("p (k c) -> p k c", k=MT), z2s[:])
            nc.gpsimd.collective_compute(
                "AllGather", ALU.bypass,
                replica_groups=[list(range(ncores))],
                ins=[ag_in[:].opt()], outs=[ag_out[:].opt()],
            )

            # y_nT = featT - 0.5*sigmoid(pre1T)   (T layout -> next-layer lhsT)
            yT2 = st.tile([P, KF, B], bf16, name="yT2", tag="yT2")
            pre1T = sig_mm0("t12", True)
            for mt in range(KF):
                sg = wk.tile([P, B], bf16, name="sg_t", tag="sg_t")
                nc.scalar.activation(sg[:], pre1T[mt][:], AF.Sigmoid)
                nc.vector.scalar_tensor_tensor(
                    yT2[:, mt, :], sg[:], -LAM3, fT0[:, mt, :], op0=ALU.mult, op1=ALU.add
                )

            # local transposes: z1T2/z2T2 = (z_n block).T for layer-1 factor lhsT
            z1T2 = st.tile([P, KF, B], bf16, name="z1T2", tag="z1T2")
            z2T2 = st.tile([P, KF, B], bf16, name="z2T2", tag="z2T2")
            for (src, dst) in [(z1s, z1T2), (z2s, z2T2)]:
                for jf in range(KF):
                    psum = ps.tile([P, B], bf16, name="psum_tr", tag="psum")
                    for it in range(MT):
                        nc.tensor.transpose(
                            psum[:, it * P:(it + 1) * P],
                            src[:, it, jf * P:(jf + 1) * P],
                            ident_sb[:],
                        )
                    nc.vector.tensor_copy(dst[:, jf, :], psum[:])

            # ======================= LAYER 1 =======================
            xTs1 = {"y": yT2, "z1": z1T2, "z2": z2T2}
            temps1 = [("t12", "y"), ("t24", "z1"), ("t34", "z2")]
            S1 = [[("y", "t24", -1.0), ("z1", "t24", EPS), ("z2", "t24", (1 - EPS)),
                   ("y", "t34", -1.0), ("z1", "t34", EPS), ("z2", "t34", (1 - EPS))],
                  [("y", "t12", 1.0), ("z1", "t12", -EPS), ("z2", "t12", -(1 - EPS))]]
            xw1, xwT1, sig_mm1, adj_mm1 = layer(xTs1, temps1, S1, 1)

            fin = {}
            for name in ("y", "z1", "z2"):
                fin[name] = st.tile([P, MT, f], bf16, name=f"fin_{name}", tag=f"fin_{name}")

            # adj matmuls first (depend on the AllGather, not the AllReduce)
            adj_out1 = []
            for ai, (dadj, zi) in enumerate(((dadjT, 0), (dadj1T, 1))):
                psums = adj_mm1(dadj, lambda j, z=zi: ag_strip(z, j))
                ao = st.tile([P, MT, f], bf16, name=f"adj_out{ai}", tag=f"adj_out{ai}")
                for mt in range(MT):
                    nc.vector.tensor_copy(ao[:, mt, :], psums[mt][:])
                adj_out1.append(ao)

            for (tname, ao, dst, cf) in [
                ("t24", adj_out1[0], fin["z1"], EPS * LAM3 / LAM1),
                ("t34", adj_out1[1], fin["z2"], (1 - EPS) * LAM3 / LAM2),
            ]:
                pre = sig_mm1(tname, False)
                for mt in range(MT):
                    sg = wk.tile([P, f], f32, name="sg_f", tag="sg_f")
                    nc.scalar.activation(sg[:], pre[mt][:], AF.Sigmoid)
                    nc.vector.scalar_tensor_tensor(
                        dst[:, mt, :], sg[:], -cf, ao[:, mt, :],
                        op0=ALU.mult, op1=ALU.add,
                    )

            pre1 = sig_mm1("t12", False)
            for mt in range(MT):
                sg = wk.tile([P, f], f32, name="sg_f", tag="sg_f")
                nc.scalar.activation(sg[:], pre1[mt][:], AF.Sigmoid)
                nc.vector.scalar_tensor_tensor(
                    fin["y"][:, mt, :], sg[:], -LAM3, featblk_sb[:, mt, :],
                    op0=ALU.mult, op1=ALU.add,
                )

            # ============== FINAL: l2norm, p.T, logits, log_softmax ==============
            pT = st.tile([P, 3 * KF, B], bf16, name="pT", tag="pT")
            for xi, name in [(1, "z1"), (2, "z2"), (0, "y")]:
                x = fin[name]
                normed = wk.tile([P, MT, f], bf16, name="normed", tag="normed", bufs=1)
                for rc in range(MT):
                    sq = wk.tile([P, f], f32, name="sq", tag="lg")
                    ssq = wk.tile([P, 1], f32, name="ssq", tag="ssq")
                    nc.scalar.activation(sq[:], x[:, rc, :], AF.Square, accum_out=ssq[:])
                    nrm = wk.tile([P, 1], f32, name="nrm", tag="nrm")
                    nc.scalar.activation(nrm[:], ssq[:], AF.Sqrt)
                    nc.vector.tensor_scalar_max(nrm[:], nrm[:], 1e-12)
                    rn = wk.tile([P, 1], f32, name="rn", tag="rn")
                    nc.vector.reciprocal(rn[:], nrm[:])
                    nc.scalar.activation(normed[:, rc, :], x[:, rc, :], AF.Copy, scale=rn[:])
                for jf in range(KF):
                    psum = ps.tile([P, B], bf16, name="psum_tr", tag="psum")
                    for it in range(MT):
                        nc.tensor.transpose(
                            psum[:, it * P:(it + 1) * P],
                            normed[:, it, jf * P:(jf + 1) * P],
                            ident_sb[:],
                        )
                    nc.vector.tensor_copy(pT[:, xi * KF + jf, :], psum[:])

            w2strips = []
            for j in range(3 * KF // SS):
                w2s = sp.tile([P, SS, f], bf16, name="w2s", tag="astrip")
                nc.sync.dma_start(w2s[:], dw2T[:, j * SS:(j + 1) * SS, :])
                w2strips.append(w2s)
            for mt in range(MT):
                psum = ps.tile([P, f], f32, name="psum", tag="psum")
                for kc in range(3 * KF):
                    nc.tensor.matmul(psum[:], pT[:, kc, mt * P:(mt + 1) * P],
                                     w2strips[kc // SS][:, kc % SS, :],
                                     start=(kc == 0), stop=False)
                nc.tensor.matmul(psum[:], ones_sb[:], w2b_sb[:], start=False, stop=True)
                # log_softmax (fp32)
                L = wk.tile([P, f], f32, name="lg", tag="lg")
                nc.vector.tensor_copy(L[:], psum[:])
                nmx = wk.tile([P, 1], f32, name="nmx", tag="nmx")
                nc.vector.tensor_reduce(out=nmx[:], in_=L[:], op=ALU.max,
                                        axis=mybir.AxisListType.X, negate=True)
                ex = wk.tile([P, f], f32, name="ex", tag="ex")
                se = wk.tile([P, 1], f32, name="se", tag="se")
                nc.scalar.activation(ex[:], L[:], AF.Exp, bias=nmx[:], accum_out=se[:])
                lse = wk.tile([P, 1], f32, name="lse", tag="lse")
                nc.scalar.activation(lse[:], se[:], AF.Ln)
                b2 = wk.tile([P, 1], f32, name="b2", tag="b2")
                nc.vector.tensor_sub(b2[:], nmx[:], lse[:])
                lo = wk.tile([P, f], f32, name="lo", tag="lo")
                nc.scalar.activation(lo[:], L[:], AF.Identity, bias=b2[:])
                nc.sync.dma_start(dout[mt * P:(mt + 1) * P, :], lo[:])

    split_waits(nc, maxw=1)
    return nc


def _img(a):
    """[k*128, C] row-major -> SBUF image [128, k*C]."""
    R, C = a.shape
    k = R // P
    return np.ascontiguousarray(
        a.reshape(k, P, C).transpose(1, 0, 2).reshape(P, k * C))


def prep_inputs(feat, adj, adj1, y, wy, w2_w, w2_b, n=N_FULL, ncores=NC):
    """Host-side sharding / transposition / casting / imaging. Returns in_maps."""
    B = n // ncores
    wy64 = np.asarray(wy, np.float64)
    WS = wy64 @ wy64.T + wy64.T @ wy64
    wy_img = _img(np.asarray(wy, np.float32).astype(BF))
    WS_img = _img(WS.astype(np.float32).astype(BF))
    w2T_img = _img(np.ascontiguousarray(np.asarray(w2_w, np.float32).T).astype(BF))
    w2b_bf = np.asarray(w2_b, np.float32).reshape(1, -1).astype(BF)
    ident = np.eye(P, dtype=np.float32).astype(BF)
    featfull_img = _img(np.asarray(feat, np.float32).astype(BF))
    feat_bf = np.asarray(feat, np.float32).astype(BF)
    in_maps = []
    for c in range(ncores):
        r0, r1 = c * B, (c + 1) * B
        in_maps.append({
            "yT": _img(np.ascontiguousarray(np.asarray(y[r0:r1], np.float32).T).astype(BF)),
            "featT": _img(np.ascontiguousarray(np.asarray(feat[r0:r1], np.float32).T).astype(BF)),
            "featblk": _img(feat_bf[r0:r1]),
            "featfull": featfull_img,
            "adjT": _img(np.ascontiguousarray(np.asarray(adj[r0:r1], np.float32).T).astype(BF)),
            "adj1T": _img(np.ascontiguousarray(np.asarray(adj1[r0:r1], np.float32).T).astype(BF)),
            "wy": wy_img,
            "WS": WS_img,
            "w2T": w2T_img,
            "w2b": w2b_bf,
            "ident": ident,
        })
    return in_maps


_CACHED_NC = None


def kernel(feat, adj, adj1, y, wy, w2_w, w2_b):
    global _CACHED_NC
    if _CACHED_NC is None:
        _CACHED_NC = build()
    in_maps = prep_inputs(feat, adj, adj1, y, wy, w2_w, w2_b)
    res = run_bass_kernel_spmd(_CACHED_NC, in_maps, core_ids=list(range(NC)))
    return np.concatenate([res.results[c]["out"] for c in range(NC)], axis=0)
